# revision 27
# baseline (speedup 1.0000x reference)
"""ACT-LSTM (adaptive computation time) forward pass on 8 TRN2 NeuronCores.

Strategy
--------
Pure data parallel: batch (32768 rows) is split into 8 shards of 4096 rows;
every core runs the full recurrence on its shard with replicated weights.

The halting dynamics of this network guarantee p_sum crosses the 1-eps
threshold for every row within 3 iterations (per-iteration halt prob is
sigmoid(..) >= ~0.47, so after 3 steps p >= ~1.4 >> 0.999).  The main kernel
therefore runs T=3 iterations and also reports the number of still-active
rows; in the (practically impossible) event rows remain active, a full
32-iteration kernel is built lazily and used instead.

On-chip layout: every [rows, H] tensor is stored transposed as a grid of
[128, 512] tiles (H on partitions, rows on free dim), which makes the whole
recurrence transpose-free: matmuls are weight-stationary
(out[n, r] = sum_k W[k, n] * state_T[k, r]), gate activations read PSUM
directly, and the elementwise cell/state updates are layout-agnostic.
Row-vector state (p_sum / active / acc) lives as [128, 32] tiles
(row = 128*col + partition), produced directly in that layout by the final
head matmuls (lhsT = hidden activations, N=1).

All matmul operands are bf16 (fp32 PSUM accumulate); the halting vector
chain is fp32.  Host-side simulation vs the fp32 reference: max elementwise
relative error ~3e-4.
"""

import numpy as np
import ml_dtypes

NCORES = 8
B = 32768
BS = B // NCORES          # rows per core
H = 512
KT = H // 128             # 4 partition tiles of the hidden dim
RC = 512                  # row-chunk (matmul moving free dim / PSUM bank)
NCH = BS // RC            # 8 row chunks
NSUB = RC // 128          # 4 sub-chunks of 128 rows per chunk
NCOL = NCH * NSUB         # 32 columns of the [128, 32] row-vector tiles
MAX_ITER = 32
THR = float(np.float32(1.0) - np.float32(1e-3))
GATES = ("i", "f", "c", "o")

_cache = {}


def _make_tc_class():
    import concourse.mybir as mybir
    import concourse.tile as tile
    from concourse.vector_clock import ScopedClock

    class _TC(tile.TileContext):
        """TileContext adjusted for this toolchain's walrus, which encodes at
        most one sync wait and one sem update per instruction (and none on
        Drain).  Extra syncs are spread over adjacent no-ops on the same
        engine (safe: engine streams issue in order), and the exit barrier
        (whose eq-waits are unencodable) is replaced by explicit per-sem
        wait_ge instructions + plain drains.  Semaphores start zeroed at NEFF
        load and we load freshly per run, so no exit sem-clear is needed."""

        def _drain_and_barrier(self, tick_clock, wait_clock):
            nc = self.nc
            probe = mybir.InstNoOp(name="tile_exit_wait_probe", ins=[], outs=[])
            probe.engine = mybir.EngineType.SP
            wait_clock.add_sem_waits(
                probe, ScopedClock({None: tick_clock.global_clock})
            )
            handles = {h.name: h for h in wait_clock.sems.allocated().values()}
            si = probe.sync_info
            if si is not None:
                for w in si.on_wait:
                    nc.sync.wait_ge(handles[w.ant_name], w.wait_value)
            for _, eng in nc.engines.items():
                eng.drain()
            popped = nc._tile_sem_poison_stack.pop()
            assert popped is self._sem_poison

        def _lower_ordered_insts(self, ordered):
            nc = self.nc

            def mknop(engine, wait=None, update=None):
                n = mybir.InstNoOp(
                    name=nc.get_next_instruction_name(), ins=[], outs=[]
                )
                n.engine = engine
                n.bass_nofuse = True
                n.sync_info = mybir.SyncInfo(
                    on_wait=[wait] if wait is not None else [],
                    on_update=[update] if update is not None else [],
                )
                return n

            for bb, insts in ordered.items():
                out = []
                for inst in insts:
                    si = inst.sync_info
                    if si is None:
                        out.append(inst)
                        continue
                    waits = list(si.on_wait)
                    ups = list(si.on_update)
                    for w in waits:
                        assert w.wait_mode == "sem-ge-imm", w
                    if isinstance(inst, mybir.InstDrain):
                        pre, keepw = waits, []
                        keepu, post = [], ups
                    else:
                        pre, keepw = waits[:-1], waits[-1:]
                        keepu, post = ups[:1], ups[1:]
                    if pre or post:
                        for w in pre:
                            out.append(mknop(inst.engine, wait=w))
                        inst.sync_info = mybir.SyncInfo(
                            on_wait=keepw, on_update=keepu
                        )
                        out.append(inst)
                        for u in post:
                            out.append(mknop(inst.engine, update=u))
                    else:
                        out.append(inst)
                ordered[bb] = out
            super()._lower_ordered_insts(ordered)

    return _TC


def _build(T):
    """Build the Bass graph for T recurrence iterations (is_last at t==31)."""
    import concourse.bass as bass
    import concourse.mybir as mybir

    dtf = mybir.dt.float32
    dtb = mybir.dt.bfloat16
    AF = mybir.ActivationFunctionType
    OP = mybir.AluOpType
    TC = _make_tc_class()

    nc = bass.Bass()

    xa_d = nc.declare_dram_parameter("xa", [3, BS], dtb, isOutput=False)
    wh_d = {g: nc.declare_dram_parameter(f"wh_{g}", [H, H], dtb, isOutput=False)
            for g in GATES}
    wxb_d = {g: nc.declare_dram_parameter(f"wxb_{g}", [3, H], dtb, isOutput=False)
             for g in GATES}
    w1o_d = nc.declare_dram_parameter("w1o", [H, 128], dtb, isOutput=False)
    w1h_d = nc.declare_dram_parameter("w1h", [H, 128], dtb, isOutput=False)
    b1o_d = nc.declare_dram_parameter("b1o", [128, 1], dtf, isOutput=False)
    b1h_d = nc.declare_dram_parameter("b1h", [128, 1], dtf, isOutput=False)
    w23_d = nc.declare_dram_parameter("w23", [128, 1], dtb, isOutput=False)
    wh2_d = nc.declare_dram_parameter("wh2", [128, 1], dtb, isOutput=False)
    b23_d = nc.declare_dram_parameter("b23v", [128, 1], dtf, isOutput=False)
    bh2_d = nc.declare_dram_parameter("bh2v", [128, 1], dtf, isOutput=False)
    acc_d = nc.declare_dram_parameter("acc_out", [BS], dtf, isOutput=True)
    act_d = nc.declare_dram_parameter("act_out", [128, 1], dtf, isOutput=True)

    with TC(nc) as tc:
        with (
            tc.tile_pool(name="persist", bufs=1) as pp,
            tc.tile_pool(name="trans", bufs=2) as tp,
            tc.tile_pool(name="ps_main", bufs=8, space="PSUM") as ps_main,
            tc.tile_pool(name="ps_vec", bufs=1, space="PSUM") as ps_vec,
        ):
            # ---- load weights / inputs ----
            wh = {}
            for g in GATES:
                for k in range(KT):
                    t = pp.tile([128, H], dtb, name=f"wh_{g}{k}", tag=f"wh_{g}{k}")
                    nc.sync.dma_start(t[:], wh_d[g][k * 128:(k + 1) * 128, :])
                    wh[(g, k)] = t
            # x-projection operands replicated at partition offsets 0/32/64/96
            # so the four gates' K=3 matmuls run concurrently in distinct
            # PE row groups (tile_position row tiling)
            xa_rep = pp.tile([128, BS], dtb, name="xa_rep", tag="xa_rep")
            wxb_rep = pp.tile([128, H], dtb, name="wxb_rep", tag="wxb_rep")
            for gi, g in enumerate(GATES):
                nc.sync.dma_start(xa_rep[32 * gi:32 * gi + 3, :], xa_d[:])
                nc.sync.dma_start(wxb_rep[32 * gi:32 * gi + 3, :], wxb_d[g][:])
            w1o, w1h = [], []
            for k in range(KT):
                a = pp.tile([128, 128], dtb, name=f"w1o{k}", tag=f"w1o{k}")
                nc.sync.dma_start(a[:], w1o_d[k * 128:(k + 1) * 128, :])
                w1o.append(a)
                b = pp.tile([128, 128], dtb, name=f"w1h{k}", tag=f"w1h{k}")
                nc.sync.dma_start(b[:], w1h_d[k * 128:(k + 1) * 128, :])
                w1h.append(b)
            b1o = pp.tile([128, 1], dtf, name="b1o", tag="b1o")
            nc.sync.dma_start(b1o[:], b1o_d[:])
            b1h = pp.tile([128, 1], dtf, name="b1h", tag="b1h")
            nc.sync.dma_start(b1h[:], b1h_d[:])
            w23 = pp.tile([128, 1], dtb, name="w23", tag="w23")
            nc.sync.dma_start(w23[:], w23_d[:])
            wh2 = pp.tile([128, 1], dtb, name="wh2", tag="wh2")
            nc.sync.dma_start(wh2[:], wh2_d[:])
            b23 = pp.tile([128, 1], dtf, name="b23", tag="b23")
            nc.sync.dma_start(b23[:], b23_d[:])
            bh2 = pp.tile([128, 1], dtf, name="bh2", tag="bh2")
            nc.sync.dma_start(bh2[:], bh2_d[:])

            # ---- persistent recurrent state ----
            st = [[pp.tile([128, RC], dtb, name=f"st{k}_{c}", tag=f"st{k}_{c}") for c in range(NCH)]
                  for k in range(KT)]
            cl = [[pp.tile([128, RC], dtb, name=f"cl{k}_{c}", tag=f"cl{k}_{c}") for c in range(NCH)]
                  for k in range(KT)]
            p_sum = pp.tile([128, NCOL], dtf, name="p_sum", tag="p_sum")
            active = pp.tile([128, NCOL], dtf, name="active", tag="active")
            acc = pp.tile([128, NCOL], dtf, name="acc", tag="acc")
            nc.vector.memset(p_sum[:], 0.0)
            nc.vector.memset(active[:], 1.0)
            nc.vector.memset(acc[:], 0.0)

            AFG = {"i": AF.Sigmoid, "f": AF.Sigmoid, "c": AF.Tanh, "o": AF.Sigmoid}

            def unit_gates(c, t):
                """Gate matmuls + activations + cell/state update for one
                (row-chunk, iteration)."""
                cs = slice(c * RC, (c + 1) * RC)
                gates_t = GATES if t > 0 else ("i", "c", "o")
                for n in range(KT):
                    ns = slice(n * 128, (n + 1) * 128)
                    psg = {}
                    for g in gates_t:
                        gi = GATES.index(g)
                        rs = slice(32 * gi, 32 * gi + 3)
                        ps = ps_main.tile([128, RC], dtf, name="ps", tag="ps")
                        nc.tensor.matmul(
                            ps[:], wxb_rep[rs, ns], xa_rep[rs, cs],
                            start=True, stop=(t == 0),
                            tile_position=(32 * gi, 0),
                        )
                        psg[g] = ps
                    gsb = {}
                    for g in gates_t:
                        ps = psg[g]
                        if t > 0:
                            for k in range(KT):
                                nc.tensor.matmul(
                                    ps[:], wh[(g, k)][:, ns], st[k][c][:],
                                    start=False, stop=(k == KT - 1),
                                )
                        gt = tp.tile([128, RC], dtb, name=f"g_{g}", tag=f"g_{g}")
                        nc.scalar.activation(gt[:], ps[:], AFG[g])
                        gsb[g] = gt
                    if t == 0:
                        nc.vector.tensor_mul(cl[n][c][:], gsb["i"][:], gsb["c"][:])
                    else:
                        t1 = tp.tile([128, RC], dtb, name="t1", tag="t1")
                        nc.vector.tensor_mul(t1[:], gsb["f"][:], cl[n][c][:])
                        t2 = tp.tile([128, RC], dtb, name="t2", tag="t2")
                        nc.vector.tensor_mul(t2[:], gsb["i"][:], gsb["c"][:])
                        nc.vector.tensor_add(cl[n][c][:], t1[:], t2[:])
                    tnc = tp.tile([128, RC], dtb, name="tnc", tag="tnc")
                    nc.scalar.activation(tnc[:], cl[n][c][:], AF.Tanh)
                    nc.vector.tensor_mul(st[n][c][:], gsb["o"][:], tnc[:])

            def unit_heads(c, t):
                """Output/halt heads + this chunk's slice of the halting
                chain.  Emitted one unit after unit_gates(c, t) so the PE
                stream never waits on the gate ACT->DVE chain."""
                vs = slice(c * NSUB, (c + 1) * NSUB)
                h1p = ps_main.tile([128, RC], dtf, name="ps", tag="ps")
                for k in range(KT):
                    nc.tensor.matmul(h1p[:], w1o[k][:], st[k][c][:],
                                     start=(k == 0), stop=(k == KT - 1))
                h1 = tp.tile([128, RC], dtb, name="h1", tag="h1")
                nc.vector.tensor_scalar(
                    h1[:], h1p[:], b1o[:, 0:1], 0.0, OP.add, OP.max
                )
                hhp = ps_main.tile([128, RC], dtf, name="ps", tag="ps")
                for k in range(KT):
                    nc.tensor.matmul(hhp[:], w1h[k][:], st[k][c][:],
                                     start=(k == 0), stop=(k == KT - 1))
                hh = tp.tile([128, RC], dtb, name="hh", tag="hh")
                nc.vector.tensor_scalar(
                    hh[:], hhp[:], b1h[:, 0:1], 0.0, OP.add, OP.max
                )
                # the N=1 head outputs land in spare columns of the already
                # consumed h1p/hhp psum tiles — no extra PSUM banks needed
                ovp = h1p[:, RC - NSUB:RC]
                hvp = hhp[:, RC - NSUB:RC]
                for s in range(NSUB):
                    ss = slice(s * 128, (s + 1) * 128)
                    nc.tensor.matmul(ovp[:, s:s + 1], h1[:, ss], w23[:],
                                     start=True, stop=True)
                    nc.tensor.matmul(hvp[:, s:s + 1], hh[:, ss], wh2[:],
                                     start=True, stop=True)

                # halting chain on this chunk's 4 columns (fp32)
                outv = tp.tile([128, NSUB], dtf, name="outv", tag="outv")
                nc.scalar.activation(outv[:], ovp[:], AF.Sigmoid, bias=b23[:, 0:1])
                halt = tp.tile([128, NSUB], dtf, name="halt", tag="halt")
                nc.scalar.activation(halt[:], hvp[:], AF.Sigmoid, bias=bh2[:, 0:1])
                halt_m = tp.tile([128, NSUB], dtf, name="halt_m", tag="halt_m")
                nc.vector.tensor_mul(halt_m[:], halt[:], active[:, vs])
                p_new = tp.tile([128, NSUB], dtf, name="p_new", tag="p_new")
                nc.vector.tensor_add(p_new[:], p_sum[:, vs], halt_m[:])
                fin = tp.tile([128, NSUB], dtf, name="fin", tag="fin")
                if t == MAX_ITER - 1:
                    nc.vector.memset(fin[:], 1.0)
                else:
                    nc.vector.tensor_single_scalar(fin[:], p_new[:], THR, OP.is_ge)
                adj = tp.tile([128, NSUB], dtf, name="adj", tag="adj")
                nc.vector.tensor_mul(adj[:], active[:, vs], fin[:])
                negt = tp.tile([128, NSUB], dtf, name="negt", tag="negt")
                nc.vector.scalar_tensor_tensor(
                    negt[:], p_new[:], 1.0, adj[:], OP.subtract, OP.mult
                )
                halt_adj = tp.tile([128, NSUB], dtf, name="halt_adj", tag="halt_adj")
                nc.vector.tensor_sub(halt_adj[:], halt_m[:], negt[:])
                nc.vector.tensor_sub(p_sum[:, vs], p_new[:], negt[:])
                wout = tp.tile([128, NSUB], dtf, name="wout", tag="wout")
                nc.vector.tensor_mul(wout[:], outv[:], halt_adj[:])
                nc.vector.tensor_add(acc[:, vs], acc[:, vs], wout[:])
                nc.vector.tensor_sub(active[:, vs], active[:, vs], adj[:])

            # iteration-major: keeps the distance between a chunk's state
            # write (DVE) and its next-iteration read (PE) at NCH units so
            # the in-order PE stream never stalls on it; heads trail the
            # gates by one unit for the same reason
            for t in range(T):
                for c in range(NCH):
                    unit_gates(c, t)
                    unit_heads(c, t)

            # ---- outputs ----
            # acc[p, col] is row 128*col + p; transpose 32x32 blocks so DRAM
            # write is contiguous
            accT = pp.tile([32, 128], dtf, name="accT", tag="accT")
            for b in range(4):
                nc.vector.transpose(
                    accT[0:32, b * 32:(b + 1) * 32],
                    acc[b * 32:(b + 1) * 32, 0:32],
                )
            nc.sync.dma_start(
                acc_d[:].rearrange("(a b) -> a b", a=32), accT[:]
            )
            act_red = pp.tile([128, 1], dtf, name="act_red", tag="act_red")
            nc.vector.reduce_sum(act_red[:], active[:], axis=mybir.AxisListType.X)
            nc.sync.dma_start(act_d[:], act_red[:])

    return nc


def _prep_shared(inputs):
    bf = ml_dtypes.bfloat16
    f32 = np.float32
    d = {k: np.asarray(v, dtype=f32) for k, v in inputs.items()}
    shared = {}
    for g in GATES:
        shared[f"wh_{g}"] = np.ascontiguousarray(d[f"W{g}_h"]).astype(bf)
        shared[f"wxb_{g}"] = np.ascontiguousarray(
            np.vstack([d[f"W{g}_x"], (d[f"b{g}_x"] + d[f"b{g}_h"])[None, :]])
        ).astype(bf)
    shared["w1o"] = np.ascontiguousarray(d["out_W1"]).astype(bf)
    shared["w1h"] = np.ascontiguousarray(d["halt_W1"]).astype(bf)
    shared["b1o"] = np.ascontiguousarray(d["out_b1"][:, None])
    shared["b1h"] = np.ascontiguousarray(d["halt_b1"][:, None])
    w23 = (d["out_W2"].astype(np.float64) @ d["out_W3"].astype(np.float64))
    shared["w23"] = np.ascontiguousarray(w23.astype(f32)).astype(bf)
    shared["wh2"] = np.ascontiguousarray(d["halt_W2"]).astype(bf)
    b23 = np.float32((d["out_b2"].astype(np.float64) @ d["out_W3"].astype(np.float64))[0]
                     + d["out_b3"][0])
    bh2 = np.float32(d["halt_b2"][0])
    shared["b23v"] = np.full((128, 1), b23, dtype=f32)
    shared["bh2v"] = np.full((128, 1), bh2, dtype=f32)
    x = d["x"]
    xa = np.vstack([x.T, np.ones((1, B), f32)]).astype(bf)  # [3, B]
    return shared, xa


def _run(nc, shared, xa, trace=False):
    from concourse.bass_utils import run_bass_kernel_spmd

    in_maps = []
    for i in range(NCORES):
        m = dict(shared)
        m["xa"] = np.ascontiguousarray(xa[:, i * BS:(i + 1) * BS])
        in_maps.append(m)
    return run_bass_kernel_spmd(
        nc, in_maps, core_ids=list(range(NCORES)), trace=trace
    )


def _get_nc(T):
    key = ("nc", T)
    if key not in _cache:
        _cache[key] = _build(T)
    return _cache[key]


def kernel(**inputs):
    shared, xa = _prep_shared(inputs)
    res = _run(_get_nc(3), shared, xa)
    accs = [res.results[i]["acc_out"] for i in range(NCORES)]
    n_active = sum(float(res.results[i]["act_out"].sum()) for i in range(NCORES))
    if n_active > 0.5:
        # fallback: some rows did not halt within 3 iterations — run the full
        # 32-iteration recurrence (matches the reference exactly)
        res = _run(_get_nc(MAX_ITER), shared, xa)
        accs = [res.results[i]["acc_out"] for i in range(NCORES)]
    out = np.concatenate(accs).reshape(B, 1).astype(np.float32)
    return out


# revision 28
# speedup vs baseline: 1.2108x; 1.2108x over previous
"""ACT-LSTM (adaptive computation time) forward pass on 8 TRN2 NeuronCores.

Strategy
--------
Pure data parallel: batch (32768 rows) is split into 8 shards of 4096 rows;
every core runs the full recurrence on its shard with replicated weights.

The halting dynamics of this network guarantee p_sum crosses the 1-eps
threshold for every row within 3 iterations (per-iteration halt prob is
sigmoid(..) >= ~0.47, so after 3 steps p >= ~1.4 >> 0.999).  The main kernel
therefore runs T=3 iterations and also reports the number of still-active
rows; in the (practically impossible) event rows remain active, a full
32-iteration kernel is built lazily and used instead.

On-chip layout: every [rows, H] tensor is stored transposed as a grid of
[128, 512] tiles (H on partitions, rows on free dim), which makes the whole
recurrence transpose-free: matmuls are weight-stationary
(out[n, r] = sum_k W[k, n] * state_T[k, r]), gate activations read PSUM
directly, and the elementwise cell/state updates are layout-agnostic.
Row-vector state (p_sum / active / acc) lives as [128, 32] tiles
(row = 128*col + partition), produced directly in that layout by the final
head matmuls (lhsT = hidden activations, N=1).

All matmul operands are bf16 (fp32 PSUM accumulate); the halting vector
chain is fp32.  Host-side simulation vs the fp32 reference: max elementwise
relative error ~3e-4.
"""

import numpy as np
import ml_dtypes

NCORES = 8
B = 32768
BS = B // NCORES          # rows per core
H = 512
KT = H // 128             # 4 partition tiles of the hidden dim
RC = 512                  # row-chunk (matmul moving free dim / PSUM bank)
NCH = BS // RC            # 8 row chunks
NSUB = RC // 128          # 4 sub-chunks of 128 rows per chunk
NCOL = NCH * NSUB         # 32 columns of the [128, 32] row-vector tiles
MAX_ITER = 32
THR = float(np.float32(1.0) - np.float32(1e-3))
GATES = ("i", "f", "c", "o")

_cache = {}


def _make_tc_class():
    import concourse.mybir as mybir
    import concourse.tile as tile
    from concourse.vector_clock import ScopedClock

    class _TC(tile.TileContext):
        """TileContext adjusted for this toolchain's walrus, which encodes at
        most one sync wait and one sem update per instruction (and none on
        Drain).  Extra syncs are spread over adjacent no-ops on the same
        engine (safe: engine streams issue in order), and the exit barrier
        (whose eq-waits are unencodable) is replaced by explicit per-sem
        wait_ge instructions + plain drains.  Semaphores start zeroed at NEFF
        load and we load freshly per run, so no exit sem-clear is needed."""

        def _drain_and_barrier(self, tick_clock, wait_clock):
            nc = self.nc
            probe = mybir.InstNoOp(name="tile_exit_wait_probe", ins=[], outs=[])
            probe.engine = mybir.EngineType.SP
            wait_clock.add_sem_waits(
                probe, ScopedClock({None: tick_clock.global_clock})
            )
            handles = {h.name: h for h in wait_clock.sems.allocated().values()}
            si = probe.sync_info
            if si is not None:
                for w in si.on_wait:
                    nc.sync.wait_ge(handles[w.ant_name], w.wait_value)
            for _, eng in nc.engines.items():
                eng.drain()
            popped = nc._tile_sem_poison_stack.pop()
            assert popped is self._sem_poison

        def _lower_ordered_insts(self, ordered):
            nc = self.nc

            def mknop(engine, wait=None, update=None):
                n = mybir.InstNoOp(
                    name=nc.get_next_instruction_name(), ins=[], outs=[]
                )
                n.engine = engine
                n.bass_nofuse = True
                n.sync_info = mybir.SyncInfo(
                    on_wait=[wait] if wait is not None else [],
                    on_update=[update] if update is not None else [],
                )
                return n

            for bb, insts in ordered.items():
                out = []
                for inst in insts:
                    si = inst.sync_info
                    if si is None:
                        out.append(inst)
                        continue
                    waits = list(si.on_wait)
                    ups = list(si.on_update)
                    for w in waits:
                        assert w.wait_mode == "sem-ge-imm", w
                    if isinstance(inst, mybir.InstDrain):
                        pre, keepw = waits, []
                        keepu, post = [], ups
                    else:
                        pre, keepw = waits[:-1], waits[-1:]
                        keepu, post = ups[:1], ups[1:]
                    if pre or post:
                        for w in pre:
                            out.append(mknop(inst.engine, wait=w))
                        inst.sync_info = mybir.SyncInfo(
                            on_wait=keepw, on_update=keepu
                        )
                        out.append(inst)
                        for u in post:
                            out.append(mknop(inst.engine, update=u))
                    else:
                        out.append(inst)
                ordered[bb] = out
            super()._lower_ordered_insts(ordered)

    return _TC


def _build(T):
    """Build the Bass graph for T recurrence iterations (is_last at t==31)."""
    import concourse.bass as bass
    import concourse.mybir as mybir

    dtf = mybir.dt.float32
    dtb = mybir.dt.bfloat16
    AF = mybir.ActivationFunctionType
    OP = mybir.AluOpType
    TC = _make_tc_class()

    nc = bass.Bass()

    xa_d = nc.declare_dram_parameter("xa", [3, BS], dtb, isOutput=False)
    wh_d = {g: nc.declare_dram_parameter(f"wh_{g}", [H, H], dtb, isOutput=False)
            for g in GATES}
    wxb_d = {g: nc.declare_dram_parameter(f"wxb_{g}", [3, H], dtb, isOutput=False)
             for g in GATES}
    w1o_d = nc.declare_dram_parameter("w1o", [H, 128], dtb, isOutput=False)
    w1h_d = nc.declare_dram_parameter("w1h", [H, 128], dtb, isOutput=False)
    b1o_d = nc.declare_dram_parameter("b1o", [128, 1], dtf, isOutput=False)
    b1h_d = nc.declare_dram_parameter("b1h", [128, 1], dtf, isOutput=False)
    w23_d = nc.declare_dram_parameter("w23", [128, 1], dtb, isOutput=False)
    wh2_d = nc.declare_dram_parameter("wh2", [128, 1], dtb, isOutput=False)
    b23_d = nc.declare_dram_parameter("b23v", [128, 1], dtf, isOutput=False)
    bh2_d = nc.declare_dram_parameter("bh2v", [128, 1], dtf, isOutput=False)
    acc_d = nc.declare_dram_parameter("acc_out", [BS], dtf, isOutput=True)
    act_d = nc.declare_dram_parameter("act_out", [128, 1], dtf, isOutput=True)

    with TC(nc) as tc:
        with (
            tc.tile_pool(name="persist", bufs=1) as pp,
            tc.tile_pool(name="trans", bufs=2) as tp,
            tc.tile_pool(name="ps_main", bufs=8, space="PSUM") as ps_main,
            tc.tile_pool(name="ps_vec", bufs=1, space="PSUM") as ps_vec,
        ):
            # ---- load weights / inputs ----
            wh = {}
            for g in GATES:
                for k in range(KT):
                    t = pp.tile([128, H], dtb, name=f"wh_{g}{k}", tag=f"wh_{g}{k}")
                    nc.sync.dma_start(t[:], wh_d[g][k * 128:(k + 1) * 128, :])
                    wh[(g, k)] = t
            # x-projection operands replicated at partition offsets 0/32/64/96
            # so the four gates' K=3 matmuls run concurrently in distinct
            # PE row groups (tile_position row tiling)
            xa_rep = pp.tile([128, BS], dtb, name="xa_rep", tag="xa_rep")
            wxb_rep = pp.tile([128, H], dtb, name="wxb_rep", tag="wxb_rep")
            for gi, g in enumerate(GATES):
                nc.sync.dma_start(xa_rep[32 * gi:32 * gi + 3, :], xa_d[:])
                nc.sync.dma_start(wxb_rep[32 * gi:32 * gi + 3, :], wxb_d[g][:])
            w1o, w1h = [], []
            for k in range(KT):
                a = pp.tile([128, 128], dtb, name=f"w1o{k}", tag=f"w1o{k}")
                nc.sync.dma_start(a[:], w1o_d[k * 128:(k + 1) * 128, :])
                w1o.append(a)
                b = pp.tile([128, 128], dtb, name=f"w1h{k}", tag=f"w1h{k}")
                nc.sync.dma_start(b[:], w1h_d[k * 128:(k + 1) * 128, :])
                w1h.append(b)
            b1o = pp.tile([128, 1], dtf, name="b1o", tag="b1o")
            nc.sync.dma_start(b1o[:], b1o_d[:])
            b1h = pp.tile([128, 1], dtf, name="b1h", tag="b1h")
            nc.sync.dma_start(b1h[:], b1h_d[:])
            w23 = pp.tile([128, 1], dtb, name="w23", tag="w23")
            nc.sync.dma_start(w23[:], w23_d[:])
            wh2 = pp.tile([128, 1], dtb, name="wh2", tag="wh2")
            nc.sync.dma_start(wh2[:], wh2_d[:])
            b23 = pp.tile([128, 1], dtf, name="b23", tag="b23")
            nc.sync.dma_start(b23[:], b23_d[:])
            bh2 = pp.tile([128, 1], dtf, name="bh2", tag="bh2")
            nc.sync.dma_start(bh2[:], bh2_d[:])

            # ---- persistent recurrent state ----
            st = [[pp.tile([128, RC], dtb, name=f"st{k}_{c}", tag=f"st{k}_{c}") for c in range(NCH)]
                  for k in range(KT)]
            cl = [[pp.tile([128, RC], dtb, name=f"cl{k}_{c}", tag=f"cl{k}_{c}") for c in range(NCH)]
                  for k in range(KT)]
            p_sum = pp.tile([128, NCOL], dtf, name="p_sum", tag="p_sum")
            active = pp.tile([128, NCOL], dtf, name="active", tag="active")
            acc = pp.tile([128, NCOL], dtf, name="acc", tag="acc")
            nc.vector.memset(p_sum[:], 0.0)
            nc.vector.memset(active[:], 1.0)
            nc.vector.memset(acc[:], 0.0)

            AFG = {"i": AF.Sigmoid, "f": AF.Sigmoid, "c": AF.Tanh, "o": AF.Sigmoid}

            def nt_block(c, t, n):
                """Gate matmuls + activations + cell/state update for one
                (row-chunk, iteration, H-slice) block."""
                cs = slice(c * RC, (c + 1) * RC)
                gates_t = GATES if t > 0 else ("i", "c", "o")
                ns = slice(n * 128, (n + 1) * 128)
                psg = {}
                for g in gates_t:
                    gi = GATES.index(g)
                    rs = slice(32 * gi, 32 * gi + 3)
                    ps = ps_main.tile([128, RC], dtf, name="ps", tag="ps")
                    nc.tensor.matmul(
                        ps[:], wxb_rep[rs, ns], xa_rep[rs, cs],
                        start=True, stop=(t == 0),
                        tile_position=(32 * gi, 0),
                    )
                    psg[g] = ps
                gsb = {}
                for g in gates_t:
                    ps = psg[g]
                    if t > 0:
                        for k in range(KT):
                            nc.tensor.matmul(
                                ps[:], wh[(g, k)][:, ns], st[k][c][:],
                                start=False, stop=(k == KT - 1),
                            )
                    gt = tp.tile([128, RC], dtb, name=f"g_{g}", tag=f"g_{g}")
                    nc.scalar.activation(gt[:], ps[:], AFG[g])
                    gsb[g] = gt
                if t == 0:
                    nc.vector.tensor_mul(cl[n][c][:], gsb["i"][:], gsb["c"][:])
                else:
                    t1 = tp.tile([128, RC], dtb, name="t1", tag="t1")
                    nc.vector.tensor_mul(t1[:], gsb["f"][:], cl[n][c][:])
                    t2 = tp.tile([128, RC], dtb, name="t2", tag="t2")
                    nc.vector.tensor_mul(t2[:], gsb["i"][:], gsb["c"][:])
                    nc.vector.tensor_add(cl[n][c][:], t1[:], t2[:])
                tnc = tp.tile([128, RC], dtb, name="tnc", tag="tnc")
                nc.scalar.activation(tnc[:], cl[n][c][:], AF.Tanh)
                nc.vector.tensor_mul(st[n][c][:], gsb["o"][:], tnc[:])

            def unit_heads(c, t):
                """Output/halt heads + this chunk's slice of the halting
                chain.  Emitted one unit after unit_gates(c, t) so the PE
                stream never waits on the gate ACT->DVE chain."""
                vs = slice(c * NSUB, (c + 1) * NSUB)
                h1p = ps_main.tile([128, RC], dtf, name="ps", tag="ps")
                for k in range(KT):
                    nc.tensor.matmul(h1p[:], w1o[k][:], st[k][c][:],
                                     start=(k == 0), stop=(k == KT - 1))
                h1 = tp.tile([128, RC], dtb, name="h1", tag="h1")
                nc.vector.tensor_scalar(
                    h1[:], h1p[:], b1o[:, 0:1], 0.0, OP.add, OP.max
                )
                hhp = ps_main.tile([128, RC], dtf, name="ps", tag="ps")
                for k in range(KT):
                    nc.tensor.matmul(hhp[:], w1h[k][:], st[k][c][:],
                                     start=(k == 0), stop=(k == KT - 1))
                hh = tp.tile([128, RC], dtb, name="hh", tag="hh")
                nc.vector.tensor_scalar(
                    hh[:], hhp[:], b1h[:, 0:1], 0.0, OP.add, OP.max
                )
                # the N=1 head outputs land in spare columns of the already
                # consumed h1p/hhp psum tiles — no extra PSUM banks needed
                ovp = h1p[:, RC - NSUB:RC]
                hvp = hhp[:, RC - NSUB:RC]
                for s in range(NSUB):
                    ss = slice(s * 128, (s + 1) * 128)
                    nc.tensor.matmul(ovp[:, s:s + 1], h1[:, ss], w23[:],
                                     start=True, stop=True)
                    nc.tensor.matmul(hvp[:, s:s + 1], hh[:, ss], wh2[:],
                                     start=True, stop=True)

                # halting chain on this chunk's 4 columns (fp32)
                outv = tp.tile([128, NSUB], dtf, name="outv", tag="outv")
                nc.scalar.activation(outv[:], ovp[:], AF.Sigmoid, bias=b23[:, 0:1])
                halt = tp.tile([128, NSUB], dtf, name="halt", tag="halt")
                nc.scalar.activation(halt[:], hvp[:], AF.Sigmoid, bias=bh2[:, 0:1])
                halt_m = tp.tile([128, NSUB], dtf, name="halt_m", tag="halt_m")
                nc.vector.tensor_mul(halt_m[:], halt[:], active[:, vs])
                p_new = tp.tile([128, NSUB], dtf, name="p_new", tag="p_new")
                nc.vector.tensor_add(p_new[:], p_sum[:, vs], halt_m[:])
                fin = tp.tile([128, NSUB], dtf, name="fin", tag="fin")
                if t == MAX_ITER - 1:
                    nc.vector.memset(fin[:], 1.0)
                else:
                    nc.vector.tensor_single_scalar(fin[:], p_new[:], THR, OP.is_ge)
                adj = tp.tile([128, NSUB], dtf, name="adj", tag="adj")
                nc.vector.tensor_mul(adj[:], active[:, vs], fin[:])
                negt = tp.tile([128, NSUB], dtf, name="negt", tag="negt")
                nc.vector.scalar_tensor_tensor(
                    negt[:], p_new[:], 1.0, adj[:], OP.subtract, OP.mult
                )
                halt_adj = tp.tile([128, NSUB], dtf, name="halt_adj", tag="halt_adj")
                nc.vector.tensor_sub(halt_adj[:], halt_m[:], negt[:])
                nc.vector.tensor_sub(p_sum[:, vs], p_new[:], negt[:])
                wout = tp.tile([128, NSUB], dtf, name="wout", tag="wout")
                nc.vector.tensor_mul(wout[:], outv[:], halt_adj[:])
                nc.vector.tensor_add(acc[:, vs], acc[:, vs], wout[:])
                nc.vector.tensor_sub(active[:, vs], active[:, vs], adj[:])

            # iteration-major: keeps the distance between a chunk's state
            # write (DVE) and its next-iteration read (PE) at NCH units so
            # the in-order PE stream never stalls on it; heads trail the
            # gates by one unit for the same reason
            # emission is rotated by one nt-block: each unit's heads are
            # followed in the PE stream by the NEXT unit's first nt-block,
            # i.e. the heads' wait on the last gate block's ACT->DVE state
            # chain is covered by ~20 independent matmuls
            units = [(c, t) for t in range(T) for c in range(NCH)]
            nt_block(*units[0], 0)
            for i, (c, t) in enumerate(units):
                for n in range(1, KT):
                    nt_block(c, t, n)
                if i + 1 < len(units):
                    nt_block(*units[i + 1], 0)
                unit_heads(c, t)

            # ---- outputs ----
            # acc[p, col] is row 128*col + p; transpose 32x32 blocks so DRAM
            # write is contiguous
            accT = pp.tile([32, 128], dtf, name="accT", tag="accT")
            for b in range(4):
                nc.vector.transpose(
                    accT[0:32, b * 32:(b + 1) * 32],
                    acc[b * 32:(b + 1) * 32, 0:32],
                )
            nc.sync.dma_start(
                acc_d[:].rearrange("(a b) -> a b", a=32), accT[:]
            )
            act_red = pp.tile([128, 1], dtf, name="act_red", tag="act_red")
            nc.vector.reduce_sum(act_red[:], active[:], axis=mybir.AxisListType.X)
            nc.sync.dma_start(act_d[:], act_red[:])

    return nc


def _prep_shared(inputs):
    bf = ml_dtypes.bfloat16
    f32 = np.float32
    d = {k: np.asarray(v, dtype=f32) for k, v in inputs.items()}
    shared = {}
    for g in GATES:
        shared[f"wh_{g}"] = np.ascontiguousarray(d[f"W{g}_h"]).astype(bf)
        shared[f"wxb_{g}"] = np.ascontiguousarray(
            np.vstack([d[f"W{g}_x"], (d[f"b{g}_x"] + d[f"b{g}_h"])[None, :]])
        ).astype(bf)
    shared["w1o"] = np.ascontiguousarray(d["out_W1"]).astype(bf)
    shared["w1h"] = np.ascontiguousarray(d["halt_W1"]).astype(bf)
    shared["b1o"] = np.ascontiguousarray(d["out_b1"][:, None])
    shared["b1h"] = np.ascontiguousarray(d["halt_b1"][:, None])
    w23 = (d["out_W2"].astype(np.float64) @ d["out_W3"].astype(np.float64))
    shared["w23"] = np.ascontiguousarray(w23.astype(f32)).astype(bf)
    shared["wh2"] = np.ascontiguousarray(d["halt_W2"]).astype(bf)
    b23 = np.float32((d["out_b2"].astype(np.float64) @ d["out_W3"].astype(np.float64))[0]
                     + d["out_b3"][0])
    bh2 = np.float32(d["halt_b2"][0])
    shared["b23v"] = np.full((128, 1), b23, dtype=f32)
    shared["bh2v"] = np.full((128, 1), bh2, dtype=f32)
    x = d["x"]
    xa = np.vstack([x.T, np.ones((1, B), f32)]).astype(bf)  # [3, B]
    return shared, xa


def _run(nc, shared, xa, trace=False):
    from concourse.bass_utils import run_bass_kernel_spmd

    in_maps = []
    for i in range(NCORES):
        m = dict(shared)
        m["xa"] = np.ascontiguousarray(xa[:, i * BS:(i + 1) * BS])
        in_maps.append(m)
    return run_bass_kernel_spmd(
        nc, in_maps, core_ids=list(range(NCORES)), trace=trace
    )


def _get_nc(T):
    key = ("nc", T)
    if key not in _cache:
        _cache[key] = _build(T)
    return _cache[key]


def kernel(**inputs):
    shared, xa = _prep_shared(inputs)
    res = _run(_get_nc(3), shared, xa)
    accs = [res.results[i]["acc_out"] for i in range(NCORES)]
    n_active = sum(float(res.results[i]["act_out"].sum()) for i in range(NCORES))
    if n_active > 0.5:
        # fallback: some rows did not halt within 3 iterations — run the full
        # 32-iteration recurrence (matches the reference exactly)
        res = _run(_get_nc(MAX_ITER), shared, xa)
        accs = [res.results[i]["acc_out"] for i in range(NCORES)]
    out = np.concatenate(accs).reshape(B, 1).astype(np.float32)
    return out


# revision 29
# speedup vs baseline: 1.2109x; 1.0001x over previous
"""ACT-LSTM (adaptive computation time) forward pass on 8 TRN2 NeuronCores.

Strategy
--------
Pure data parallel: batch (32768 rows) is split into 8 shards of 4096 rows;
every core runs the full recurrence on its shard with replicated weights.

The halting dynamics of this network guarantee p_sum crosses the 1-eps
threshold for every row within 3 iterations (per-iteration halt prob is
sigmoid(..) >= ~0.47, so after 3 steps p >= ~1.4 >> 0.999).  The main kernel
therefore runs T=3 iterations and also reports the number of still-active
rows; in the (practically impossible) event rows remain active, a full
32-iteration kernel is built lazily and used instead.

On-chip layout: every [rows, H] tensor is stored transposed as a grid of
[128, 512] tiles (H on partitions, rows on free dim), which makes the whole
recurrence transpose-free: matmuls are weight-stationary
(out[n, r] = sum_k W[k, n] * state_T[k, r]), gate activations read PSUM
directly, and the elementwise cell/state updates are layout-agnostic.
Row-vector state (p_sum / active / acc) lives as [128, 32] tiles
(row = 128*col + partition), produced directly in that layout by the final
head matmuls (lhsT = hidden activations, N=1).

All matmul operands are bf16 (fp32 PSUM accumulate); the halting vector
chain is fp32.  Host-side simulation vs the fp32 reference: max elementwise
relative error ~3e-4.
"""

import numpy as np
import ml_dtypes

NCORES = 8
B = 32768
BS = B // NCORES          # rows per core
H = 512
KT = H // 128             # 4 partition tiles of the hidden dim
RC = 512                  # row-chunk (matmul moving free dim / PSUM bank)
NCH = BS // RC            # 8 row chunks
NSUB = RC // 128          # 4 sub-chunks of 128 rows per chunk
NCOL = NCH * NSUB         # 32 columns of the [128, 32] row-vector tiles
MAX_ITER = 32
THR = float(np.float32(1.0) - np.float32(1e-3))
GATES = ("i", "f", "c", "o")

_cache = {}


def _make_tc_class():
    import concourse.mybir as mybir
    import concourse.tile as tile
    from concourse.vector_clock import ScopedClock

    class _TC(tile.TileContext):
        """TileContext adjusted for this toolchain's walrus, which encodes at
        most one sync wait and one sem update per instruction (and none on
        Drain).  Extra syncs are spread over adjacent no-ops on the same
        engine (safe: engine streams issue in order), and the exit barrier
        (whose eq-waits are unencodable) is replaced by explicit per-sem
        wait_ge instructions + plain drains.  Semaphores start zeroed at NEFF
        load and we load freshly per run, so no exit sem-clear is needed."""

        def _drain_and_barrier(self, tick_clock, wait_clock):
            nc = self.nc
            probe = mybir.InstNoOp(name="tile_exit_wait_probe", ins=[], outs=[])
            probe.engine = mybir.EngineType.SP
            wait_clock.add_sem_waits(
                probe, ScopedClock({None: tick_clock.global_clock})
            )
            handles = {h.name: h for h in wait_clock.sems.allocated().values()}
            si = probe.sync_info
            if si is not None:
                # only DMA-queue sems need the explicit exit wait (their
                # engine-side drain doesn't cover in-flight completions of
                # queues other engines triggered); compute-engine ticks are
                # implied by each engine running its stream to the end
                for w in si.on_wait:
                    if "DMA" in w.ant_name:
                        nc.sync.wait_ge(handles[w.ant_name], w.wait_value)
            for _, eng in nc.engines.items():
                eng.drain()
            popped = nc._tile_sem_poison_stack.pop()
            assert popped is self._sem_poison

        def _lower_ordered_insts(self, ordered):
            nc = self.nc

            def mknop(engine, wait=None, update=None):
                n = mybir.InstNoOp(
                    name=nc.get_next_instruction_name(), ins=[], outs=[]
                )
                n.engine = engine
                n.bass_nofuse = True
                n.sync_info = mybir.SyncInfo(
                    on_wait=[wait] if wait is not None else [],
                    on_update=[update] if update is not None else [],
                )
                return n

            for bb, insts in ordered.items():
                out = []
                for inst in insts:
                    si = inst.sync_info
                    if si is None:
                        out.append(inst)
                        continue
                    waits = list(si.on_wait)
                    ups = list(si.on_update)
                    for w in waits:
                        assert w.wait_mode == "sem-ge-imm", w
                    if isinstance(inst, mybir.InstDrain):
                        pre, keepw = waits, []
                        keepu, post = [], ups
                    else:
                        pre, keepw = waits[:-1], waits[-1:]
                        keepu, post = ups[:1], ups[1:]
                    if pre or post:
                        for w in pre:
                            out.append(mknop(inst.engine, wait=w))
                        inst.sync_info = mybir.SyncInfo(
                            on_wait=keepw, on_update=keepu
                        )
                        out.append(inst)
                        for u in post:
                            out.append(mknop(inst.engine, update=u))
                    else:
                        out.append(inst)
                ordered[bb] = out
            super()._lower_ordered_insts(ordered)

    return _TC


def _build(T):
    """Build the Bass graph for T recurrence iterations (is_last at t==31)."""
    import concourse.bass as bass
    import concourse.mybir as mybir

    dtf = mybir.dt.float32
    dtb = mybir.dt.bfloat16
    AF = mybir.ActivationFunctionType
    OP = mybir.AluOpType
    TC = _make_tc_class()

    nc = bass.Bass()

    xa_d = nc.declare_dram_parameter("xa", [3, BS], dtb, isOutput=False)
    wh_d = {g: nc.declare_dram_parameter(f"wh_{g}", [H, H], dtb, isOutput=False)
            for g in GATES}
    wxb_d = {g: nc.declare_dram_parameter(f"wxb_{g}", [3, H], dtb, isOutput=False)
             for g in GATES}
    w1o_d = nc.declare_dram_parameter("w1o", [H, 128], dtb, isOutput=False)
    w1h_d = nc.declare_dram_parameter("w1h", [H, 128], dtb, isOutput=False)
    b1o_d = nc.declare_dram_parameter("b1o", [128, 1], dtf, isOutput=False)
    b1h_d = nc.declare_dram_parameter("b1h", [128, 1], dtf, isOutput=False)
    w23_d = nc.declare_dram_parameter("w23", [128, 1], dtb, isOutput=False)
    wh2_d = nc.declare_dram_parameter("wh2", [128, 1], dtb, isOutput=False)
    b23_d = nc.declare_dram_parameter("b23v", [128, 1], dtf, isOutput=False)
    bh2_d = nc.declare_dram_parameter("bh2v", [128, 1], dtf, isOutput=False)
    acc_d = nc.declare_dram_parameter("acc_out", [BS], dtf, isOutput=True)
    act_d = nc.declare_dram_parameter("act_out", [128, 1], dtf, isOutput=True)

    with TC(nc) as tc:
        with (
            tc.tile_pool(name="persist", bufs=1) as pp,
            tc.tile_pool(name="trans", bufs=2) as tp,
            tc.tile_pool(name="ps_main", bufs=8, space="PSUM") as ps_main,
            tc.tile_pool(name="ps_vec", bufs=1, space="PSUM") as ps_vec,
        ):
            # ---- load weights / inputs ----
            wh = {}
            for g in GATES:
                for k in range(KT):
                    t = pp.tile([128, H], dtb, name=f"wh_{g}{k}", tag=f"wh_{g}{k}")
                    nc.sync.dma_start(t[:], wh_d[g][k * 128:(k + 1) * 128, :])
                    wh[(g, k)] = t
            # x-projection operands replicated at partition offsets 0/32/64/96
            # so the four gates' K=3 matmuls run concurrently in distinct
            # PE row groups (tile_position row tiling)
            xa_rep = pp.tile([128, BS], dtb, name="xa_rep", tag="xa_rep")
            wxb_rep = pp.tile([128, H], dtb, name="wxb_rep", tag="wxb_rep")
            for gi, g in enumerate(GATES):
                nc.sync.dma_start(xa_rep[32 * gi:32 * gi + 3, :], xa_d[:])
                nc.sync.dma_start(wxb_rep[32 * gi:32 * gi + 3, :], wxb_d[g][:])
            w1o, w1h = [], []
            for k in range(KT):
                a = pp.tile([128, 128], dtb, name=f"w1o{k}", tag=f"w1o{k}")
                nc.sync.dma_start(a[:], w1o_d[k * 128:(k + 1) * 128, :])
                w1o.append(a)
                b = pp.tile([128, 128], dtb, name=f"w1h{k}", tag=f"w1h{k}")
                nc.sync.dma_start(b[:], w1h_d[k * 128:(k + 1) * 128, :])
                w1h.append(b)
            b1o = pp.tile([128, 1], dtf, name="b1o", tag="b1o")
            nc.sync.dma_start(b1o[:], b1o_d[:])
            b1h = pp.tile([128, 1], dtf, name="b1h", tag="b1h")
            nc.sync.dma_start(b1h[:], b1h_d[:])
            w23 = pp.tile([128, 1], dtb, name="w23", tag="w23")
            nc.sync.dma_start(w23[:], w23_d[:])
            wh2 = pp.tile([128, 1], dtb, name="wh2", tag="wh2")
            nc.sync.dma_start(wh2[:], wh2_d[:])
            b23 = pp.tile([128, 1], dtf, name="b23", tag="b23")
            nc.sync.dma_start(b23[:], b23_d[:])
            bh2 = pp.tile([128, 1], dtf, name="bh2", tag="bh2")
            nc.sync.dma_start(bh2[:], bh2_d[:])

            # ---- persistent recurrent state ----
            st = [[pp.tile([128, RC], dtb, name=f"st{k}_{c}", tag=f"st{k}_{c}") for c in range(NCH)]
                  for k in range(KT)]
            cl = [[pp.tile([128, RC], dtb, name=f"cl{k}_{c}", tag=f"cl{k}_{c}") for c in range(NCH)]
                  for k in range(KT)]
            p_sum = pp.tile([128, NCOL], dtf, name="p_sum", tag="p_sum")
            active = pp.tile([128, NCOL], dtf, name="active", tag="active")
            acc = pp.tile([128, NCOL], dtf, name="acc", tag="acc")
            nc.vector.memset(p_sum[:], 0.0)
            nc.vector.memset(active[:], 1.0)
            nc.vector.memset(acc[:], 0.0)

            AFG = {"i": AF.Sigmoid, "f": AF.Sigmoid, "c": AF.Tanh, "o": AF.Sigmoid}

            def nt_block(c, t, n):
                """Gate matmuls + activations + cell/state update for one
                (row-chunk, iteration, H-slice) block."""
                cs = slice(c * RC, (c + 1) * RC)
                gates_t = GATES if t > 0 else ("i", "c", "o")
                ns = slice(n * 128, (n + 1) * 128)
                psg = {}
                for g in gates_t:
                    gi = GATES.index(g)
                    rs = slice(32 * gi, 32 * gi + 3)
                    ps = ps_main.tile([128, RC], dtf, name="ps", tag="ps")
                    nc.tensor.matmul(
                        ps[:], wxb_rep[rs, ns], xa_rep[rs, cs],
                        start=True, stop=(t == 0),
                        tile_position=(32 * gi, 0),
                    )
                    psg[g] = ps
                gsb = {}
                for g in gates_t:
                    ps = psg[g]
                    if t > 0:
                        for k in range(KT):
                            nc.tensor.matmul(
                                ps[:], wh[(g, k)][:, ns], st[k][c][:],
                                start=False, stop=(k == KT - 1),
                            )
                    gt = tp.tile([128, RC], dtb, name=f"g_{g}", tag=f"g_{g}")
                    nc.scalar.activation(gt[:], ps[:], AFG[g])
                    gsb[g] = gt
                if t == 0:
                    nc.vector.tensor_mul(cl[n][c][:], gsb["i"][:], gsb["c"][:])
                else:
                    t1 = tp.tile([128, RC], dtb, name="t1", tag="t1")
                    nc.vector.tensor_mul(t1[:], gsb["f"][:], cl[n][c][:])
                    t2 = tp.tile([128, RC], dtb, name="t2", tag="t2")
                    nc.vector.tensor_mul(t2[:], gsb["i"][:], gsb["c"][:])
                    nc.vector.tensor_add(cl[n][c][:], t1[:], t2[:])
                tnc = tp.tile([128, RC], dtb, name="tnc", tag="tnc")
                nc.scalar.activation(tnc[:], cl[n][c][:], AF.Tanh)
                nc.vector.tensor_mul(st[n][c][:], gsb["o"][:], tnc[:])

            def unit_heads(c, t):
                """Output/halt heads + this chunk's slice of the halting
                chain.  Emitted one unit after unit_gates(c, t) so the PE
                stream never waits on the gate ACT->DVE chain."""
                vs = slice(c * NSUB, (c + 1) * NSUB)
                h1p = ps_main.tile([128, RC], dtf, name="ps", tag="ps")
                for k in range(KT):
                    nc.tensor.matmul(h1p[:], w1o[k][:], st[k][c][:],
                                     start=(k == 0), stop=(k == KT - 1))
                h1 = tp.tile([128, RC], dtb, name="h1", tag="h1")
                nc.vector.tensor_scalar(
                    h1[:], h1p[:], b1o[:, 0:1], 0.0, OP.add, OP.max
                )
                hhp = ps_main.tile([128, RC], dtf, name="ps", tag="ps")
                for k in range(KT):
                    nc.tensor.matmul(hhp[:], w1h[k][:], st[k][c][:],
                                     start=(k == 0), stop=(k == KT - 1))
                hh = tp.tile([128, RC], dtb, name="hh", tag="hh")
                nc.vector.tensor_scalar(
                    hh[:], hhp[:], b1h[:, 0:1], 0.0, OP.add, OP.max
                )
                # the N=1 head outputs land in spare columns of the already
                # consumed h1p/hhp psum tiles — no extra PSUM banks needed
                ovp = h1p[:, RC - NSUB:RC]
                hvp = hhp[:, RC - NSUB:RC]
                for s in range(NSUB):
                    ss = slice(s * 128, (s + 1) * 128)
                    nc.tensor.matmul(ovp[:, s:s + 1], h1[:, ss], w23[:],
                                     start=True, stop=True)
                    nc.tensor.matmul(hvp[:, s:s + 1], hh[:, ss], wh2[:],
                                     start=True, stop=True)

                # halting chain on this chunk's 4 columns (fp32)
                outv = tp.tile([128, NSUB], dtf, name="outv", tag="outv")
                nc.scalar.activation(outv[:], ovp[:], AF.Sigmoid, bias=b23[:, 0:1])
                halt = tp.tile([128, NSUB], dtf, name="halt", tag="halt")
                nc.scalar.activation(halt[:], hvp[:], AF.Sigmoid, bias=bh2[:, 0:1])
                halt_m = tp.tile([128, NSUB], dtf, name="halt_m", tag="halt_m")
                nc.vector.tensor_mul(halt_m[:], halt[:], active[:, vs])
                p_new = tp.tile([128, NSUB], dtf, name="p_new", tag="p_new")
                nc.vector.tensor_add(p_new[:], p_sum[:, vs], halt_m[:])
                fin = tp.tile([128, NSUB], dtf, name="fin", tag="fin")
                if t == MAX_ITER - 1:
                    nc.vector.memset(fin[:], 1.0)
                else:
                    nc.vector.tensor_single_scalar(fin[:], p_new[:], THR, OP.is_ge)
                adj = tp.tile([128, NSUB], dtf, name="adj", tag="adj")
                nc.vector.tensor_mul(adj[:], active[:, vs], fin[:])
                negt = tp.tile([128, NSUB], dtf, name="negt", tag="negt")
                nc.vector.scalar_tensor_tensor(
                    negt[:], p_new[:], 1.0, adj[:], OP.subtract, OP.mult
                )
                halt_adj = tp.tile([128, NSUB], dtf, name="halt_adj", tag="halt_adj")
                nc.vector.tensor_sub(halt_adj[:], halt_m[:], negt[:])
                nc.vector.tensor_sub(p_sum[:, vs], p_new[:], negt[:])
                wout = tp.tile([128, NSUB], dtf, name="wout", tag="wout")
                nc.vector.tensor_mul(wout[:], outv[:], halt_adj[:])
                nc.vector.tensor_add(acc[:, vs], acc[:, vs], wout[:])
                nc.vector.tensor_sub(active[:, vs], active[:, vs], adj[:])

            # iteration-major: keeps the distance between a chunk's state
            # write (DVE) and its next-iteration read (PE) at NCH units so
            # the in-order PE stream never stalls on it; heads trail the
            # gates by one unit for the same reason
            # emission is rotated by one nt-block: each unit's heads are
            # followed in the PE stream by the NEXT unit's first nt-block,
            # i.e. the heads' wait on the last gate block's ACT->DVE state
            # chain is covered by ~20 independent matmuls
            units = [(c, t) for t in range(T) for c in range(NCH)]
            nt_block(*units[0], 0)
            for i, (c, t) in enumerate(units):
                for n in range(1, KT):
                    nt_block(c, t, n)
                if i + 1 < len(units):
                    nt_block(*units[i + 1], 0)
                unit_heads(c, t)

            # ---- outputs ----
            # acc[p, col] is row 128*col + p; transpose 32x32 blocks so DRAM
            # write is contiguous
            accT = pp.tile([32, 128], dtf, name="accT", tag="accT")
            for b in range(4):
                nc.vector.transpose(
                    accT[0:32, b * 32:(b + 1) * 32],
                    acc[b * 32:(b + 1) * 32, 0:32],
                )
            nc.sync.dma_start(
                acc_d[:].rearrange("(a b) -> a b", a=32), accT[:]
            )
            act_red = pp.tile([128, 1], dtf, name="act_red", tag="act_red")
            nc.vector.reduce_sum(act_red[:], active[:], axis=mybir.AxisListType.X)
            nc.sync.dma_start(act_d[:], act_red[:])

    return nc


def _prep_shared(inputs):
    bf = ml_dtypes.bfloat16
    f32 = np.float32
    d = {k: np.asarray(v, dtype=f32) for k, v in inputs.items()}
    shared = {}
    for g in GATES:
        shared[f"wh_{g}"] = np.ascontiguousarray(d[f"W{g}_h"]).astype(bf)
        shared[f"wxb_{g}"] = np.ascontiguousarray(
            np.vstack([d[f"W{g}_x"], (d[f"b{g}_x"] + d[f"b{g}_h"])[None, :]])
        ).astype(bf)
    shared["w1o"] = np.ascontiguousarray(d["out_W1"]).astype(bf)
    shared["w1h"] = np.ascontiguousarray(d["halt_W1"]).astype(bf)
    shared["b1o"] = np.ascontiguousarray(d["out_b1"][:, None])
    shared["b1h"] = np.ascontiguousarray(d["halt_b1"][:, None])
    w23 = (d["out_W2"].astype(np.float64) @ d["out_W3"].astype(np.float64))
    shared["w23"] = np.ascontiguousarray(w23.astype(f32)).astype(bf)
    shared["wh2"] = np.ascontiguousarray(d["halt_W2"]).astype(bf)
    b23 = np.float32((d["out_b2"].astype(np.float64) @ d["out_W3"].astype(np.float64))[0]
                     + d["out_b3"][0])
    bh2 = np.float32(d["halt_b2"][0])
    shared["b23v"] = np.full((128, 1), b23, dtype=f32)
    shared["bh2v"] = np.full((128, 1), bh2, dtype=f32)
    x = d["x"]
    xa = np.vstack([x.T, np.ones((1, B), f32)]).astype(bf)  # [3, B]
    return shared, xa


def _run(nc, shared, xa, trace=False):
    from concourse.bass_utils import run_bass_kernel_spmd

    in_maps = []
    for i in range(NCORES):
        m = dict(shared)
        m["xa"] = np.ascontiguousarray(xa[:, i * BS:(i + 1) * BS])
        in_maps.append(m)
    return run_bass_kernel_spmd(
        nc, in_maps, core_ids=list(range(NCORES)), trace=trace
    )


def _get_nc(T):
    key = ("nc", T)
    if key not in _cache:
        _cache[key] = _build(T)
    return _cache[key]


def kernel(**inputs):
    shared, xa = _prep_shared(inputs)
    res = _run(_get_nc(3), shared, xa)
    accs = [res.results[i]["acc_out"] for i in range(NCORES)]
    n_active = sum(float(res.results[i]["act_out"].sum()) for i in range(NCORES))
    if n_active > 0.5:
        # fallback: some rows did not halt within 3 iterations — run the full
        # 32-iteration recurrence (matches the reference exactly)
        res = _run(_get_nc(MAX_ITER), shared, xa)
        accs = [res.results[i]["acc_out"] for i in range(NCORES)]
    out = np.concatenate(accs).reshape(B, 1).astype(np.float32)
    return out


# revision 30
# speedup vs baseline: 1.2354x; 1.0202x over previous
"""ACT-LSTM (adaptive computation time) forward pass on 8 TRN2 NeuronCores.

Strategy
--------
Pure data parallel: batch (32768 rows) is split into 8 shards of 4096 rows;
every core runs the full recurrence on its shard with replicated weights.

The halting dynamics of this network guarantee p_sum crosses the 1-eps
threshold for every row within 3 iterations (per-iteration halt prob is
sigmoid(..) >= ~0.47, so after 3 steps p >= ~1.4 >> 0.999).  The main kernel
therefore runs T=3 iterations and also reports the number of still-active
rows; in the (practically impossible) event rows remain active, a full
32-iteration kernel is built lazily and used instead.

On-chip layout: every [rows, H] tensor is stored transposed as a grid of
[128, 512] tiles (H on partitions, rows on free dim), which makes the whole
recurrence transpose-free: matmuls are weight-stationary
(out[n, r] = sum_k W[k, n] * state_T[k, r]), gate activations read PSUM
directly, and the elementwise cell/state updates are layout-agnostic.
Row-vector state (p_sum / active / acc) lives as [128, 32] tiles
(row = 128*col + partition), produced directly in that layout by the final
head matmuls (lhsT = hidden activations, N=1).

All matmul operands are bf16 (fp32 PSUM accumulate); the halting vector
chain is fp32.  Host-side simulation vs the fp32 reference: max elementwise
relative error ~3e-4.
"""

import numpy as np
import ml_dtypes

NCORES = 8
B = 32768
BS = B // NCORES          # rows per core
H = 512
KT = H // 128             # 4 partition tiles of the hidden dim
RC = 512                  # row-chunk (matmul moving free dim / PSUM bank)
NCH = BS // RC            # 8 row chunks
NSUB = RC // 128          # 4 sub-chunks of 128 rows per chunk
NCOL = NCH * NSUB         # 32 columns of the [128, 32] row-vector tiles
MAX_ITER = 32
THR = float(np.float32(1.0) - np.float32(1e-3))
GATES = ("i", "f", "c", "o")

_cache = {}


def _make_tc_class():
    import concourse.mybir as mybir
    import concourse.tile as tile
    from concourse.vector_clock import ScopedClock

    class _TC(tile.TileContext):
        """TileContext adjusted for this toolchain's walrus, which encodes at
        most one sync wait and one sem update per instruction (and none on
        Drain).  Extra syncs are spread over adjacent no-ops on the same
        engine (safe: engine streams issue in order), and the exit barrier
        (whose eq-waits are unencodable) is replaced by explicit per-sem
        wait_ge instructions + plain drains.  Semaphores start zeroed at NEFF
        load and we load freshly per run, so no exit sem-clear is needed."""

        def _drain_and_barrier(self, tick_clock, wait_clock):
            nc = self.nc
            probe = mybir.InstNoOp(name="tile_exit_wait_probe", ins=[], outs=[])
            probe.engine = mybir.EngineType.SP
            wait_clock.add_sem_waits(
                probe, ScopedClock({None: tick_clock.global_clock})
            )
            handles = {h.name: h for h in wait_clock.sems.allocated().values()}
            si = probe.sync_info
            if si is not None:
                # only DMA-queue sems need the explicit exit wait (their
                # engine-side drain doesn't cover in-flight completions of
                # queues other engines triggered); compute-engine ticks are
                # implied by each engine running its stream to the end
                for w in si.on_wait:
                    if "DMA" in w.ant_name:
                        nc.sync.wait_ge(handles[w.ant_name], w.wait_value)
            for _, eng in nc.engines.items():
                eng.drain()
            popped = nc._tile_sem_poison_stack.pop()
            assert popped is self._sem_poison

        def _lower_ordered_insts(self, ordered):
            nc = self.nc

            def mknop(engine, wait=None, update=None):
                n = mybir.InstNoOp(
                    name=nc.get_next_instruction_name(), ins=[], outs=[]
                )
                n.engine = engine
                n.bass_nofuse = True
                n.sync_info = mybir.SyncInfo(
                    on_wait=[wait] if wait is not None else [],
                    on_update=[update] if update is not None else [],
                )
                return n

            for bb, insts in ordered.items():
                out = []
                for inst in insts:
                    si = inst.sync_info
                    if si is None:
                        out.append(inst)
                        continue
                    waits = list(si.on_wait)
                    ups = list(si.on_update)
                    for w in waits:
                        assert w.wait_mode == "sem-ge-imm", w
                    if isinstance(inst, mybir.InstDrain):
                        pre, keepw = waits, []
                        keepu, post = [], ups
                    else:
                        pre, keepw = waits[:-1], waits[-1:]
                        keepu, post = ups[:1], ups[1:]
                    if pre or post:
                        for w in pre:
                            out.append(mknop(inst.engine, wait=w))
                        inst.sync_info = mybir.SyncInfo(
                            on_wait=keepw, on_update=keepu
                        )
                        out.append(inst)
                        for u in post:
                            out.append(mknop(inst.engine, update=u))
                    else:
                        out.append(inst)
                ordered[bb] = out
            super()._lower_ordered_insts(ordered)

    return _TC


def _build(T):
    """Build the Bass graph for T recurrence iterations (is_last at t==31)."""
    import concourse.bass as bass
    import concourse.mybir as mybir

    dtf = mybir.dt.float32
    dtb = mybir.dt.bfloat16
    AF = mybir.ActivationFunctionType
    OP = mybir.AluOpType
    TC = _make_tc_class()

    nc = bass.Bass()

    xa_d = nc.declare_dram_parameter("xa", [3, BS], dtb, isOutput=False)
    wh_d = {g: nc.declare_dram_parameter(f"wh_{g}", [H, H], dtb, isOutput=False)
            for g in GATES}
    wxb_d = {g: nc.declare_dram_parameter(f"wxb_{g}", [3, H], dtb, isOutput=False)
             for g in GATES}
    w1o_d = nc.declare_dram_parameter("w1o", [H, 128], dtb, isOutput=False)
    w1h_d = nc.declare_dram_parameter("w1h", [H, 128], dtb, isOutput=False)
    b1o_d = nc.declare_dram_parameter("b1o", [128, 1], dtf, isOutput=False)
    b1h_d = nc.declare_dram_parameter("b1h", [128, 1], dtf, isOutput=False)
    w23_d = nc.declare_dram_parameter("w23", [128, 1], dtb, isOutput=False)
    wh2_d = nc.declare_dram_parameter("wh2", [128, 1], dtb, isOutput=False)
    b23_d = nc.declare_dram_parameter("b23v", [128, 1], dtf, isOutput=False)
    bh2_d = nc.declare_dram_parameter("bh2v", [128, 1], dtf, isOutput=False)
    acc_d = nc.declare_dram_parameter("acc_out", [BS], dtf, isOutput=True)
    act_d = nc.declare_dram_parameter("act_out", [128, 1], dtf, isOutput=True)

    with TC(nc) as tc:
        with (
            tc.tile_pool(name="persist", bufs=1) as pp,
            tc.tile_pool(name="trans", bufs=2) as tp,
            tc.tile_pool(name="ps_main", bufs=8, space="PSUM") as ps_main,
            tc.tile_pool(name="ps_vec", bufs=1, space="PSUM") as ps_vec,
        ):
            # ---- load weights / inputs ----
            # x-projection operands first: t0 only needs these (the 2.1MB of
            # hidden weights are loaded last, overlapped with t0 compute).
            # They are replicated at partition offsets 0/32/64/96 so the four
            # gates' K=3 matmuls run concurrently in distinct PE row groups
            # (tile_position row tiling).
            xa_rep = pp.tile([128, BS], dtb, name="xa_rep", tag="xa_rep")
            wxb_rep = pp.tile([128, H], dtb, name="wxb_rep", tag="wxb_rep")
            for gi, g in enumerate(GATES):
                nc.sync.dma_start(xa_rep[32 * gi:32 * gi + 3, :], xa_d[:])
                nc.sync.dma_start(wxb_rep[32 * gi:32 * gi + 3, :], wxb_d[g][:])
            w1o, w1h = [], []
            for k in range(KT):
                a = pp.tile([128, 128], dtb, name=f"w1o{k}", tag=f"w1o{k}")
                nc.sync.dma_start(a[:], w1o_d[k * 128:(k + 1) * 128, :])
                w1o.append(a)
                b = pp.tile([128, 128], dtb, name=f"w1h{k}", tag=f"w1h{k}")
                nc.sync.dma_start(b[:], w1h_d[k * 128:(k + 1) * 128, :])
                w1h.append(b)
            b1o = pp.tile([128, 1], dtf, name="b1o", tag="b1o")
            nc.sync.dma_start(b1o[:], b1o_d[:])
            b1h = pp.tile([128, 1], dtf, name="b1h", tag="b1h")
            nc.sync.dma_start(b1h[:], b1h_d[:])
            w23 = pp.tile([128, 1], dtb, name="w23", tag="w23")
            nc.sync.dma_start(w23[:], w23_d[:])
            wh2 = pp.tile([128, 1], dtb, name="wh2", tag="wh2")
            nc.sync.dma_start(wh2[:], wh2_d[:])
            b23 = pp.tile([128, 1], dtf, name="b23", tag="b23")
            nc.sync.dma_start(b23[:], b23_d[:])
            bh2 = pp.tile([128, 1], dtf, name="bh2", tag="bh2")
            nc.sync.dma_start(bh2[:], bh2_d[:])
            wh = {}
            for g in GATES:
                for k in range(KT):
                    t = pp.tile([128, H], dtb, name=f"wh_{g}{k}", tag=f"wh_{g}{k}")
                    nc.sync.dma_start(t[:], wh_d[g][k * 128:(k + 1) * 128, :])
                    wh[(g, k)] = t

            # ---- persistent recurrent state ----
            st = [[pp.tile([128, RC], dtb, name=f"st{k}_{c}", tag=f"st{k}_{c}") for c in range(NCH)]
                  for k in range(KT)]
            cl = [[pp.tile([128, RC], dtb, name=f"cl{k}_{c}", tag=f"cl{k}_{c}") for c in range(NCH)]
                  for k in range(KT)]
            p_sum = pp.tile([128, NCOL], dtf, name="p_sum", tag="p_sum")
            active = pp.tile([128, NCOL], dtf, name="active", tag="active")
            acc = pp.tile([128, NCOL], dtf, name="acc", tag="acc")
            nc.vector.memset(p_sum[:], 0.0)
            nc.vector.memset(active[:], 1.0)
            nc.vector.memset(acc[:], 0.0)

            AFG = {"i": AF.Sigmoid, "f": AF.Sigmoid, "c": AF.Tanh, "o": AF.Sigmoid}

            def nt_block(c, t, n):
                """Gate matmuls + activations + cell/state update for one
                (row-chunk, iteration, H-slice) block."""
                cs = slice(c * RC, (c + 1) * RC)
                gates_t = GATES if t > 0 else ("i", "c", "o")
                ns = slice(n * 128, (n + 1) * 128)
                psg = {}
                for g in gates_t:
                    gi = GATES.index(g)
                    rs = slice(32 * gi, 32 * gi + 3)
                    ps = ps_main.tile([128, RC], dtf, name="ps", tag="ps")
                    nc.tensor.matmul(
                        ps[:], wxb_rep[rs, ns], xa_rep[rs, cs],
                        start=True, stop=(t == 0),
                        tile_position=(32 * gi, 0),
                    )
                    psg[g] = ps
                gsb = {}
                for g in gates_t:
                    ps = psg[g]
                    if t > 0:
                        for k in range(KT):
                            nc.tensor.matmul(
                                ps[:], wh[(g, k)][:, ns], st[k][c][:],
                                start=False, stop=(k == KT - 1),
                            )
                    gt = tp.tile([128, RC], dtb, name=f"g_{g}", tag=f"g_{g}")
                    nc.scalar.activation(gt[:], ps[:], AFG[g])
                    gsb[g] = gt
                if t == 0:
                    nc.vector.tensor_mul(cl[n][c][:], gsb["i"][:], gsb["c"][:])
                else:
                    t1 = tp.tile([128, RC], dtb, name="t1", tag="t1")
                    nc.vector.tensor_mul(t1[:], gsb["f"][:], cl[n][c][:])
                    t2 = tp.tile([128, RC], dtb, name="t2", tag="t2")
                    nc.vector.tensor_mul(t2[:], gsb["i"][:], gsb["c"][:])
                    nc.vector.tensor_add(cl[n][c][:], t1[:], t2[:])
                tnc = tp.tile([128, RC], dtb, name="tnc", tag="tnc")
                nc.scalar.activation(tnc[:], cl[n][c][:], AF.Tanh)
                nc.vector.tensor_mul(st[n][c][:], gsb["o"][:], tnc[:])

            def unit_heads(c, t):
                """Output/halt heads + this chunk's slice of the halting
                chain.  Emitted one unit after unit_gates(c, t) so the PE
                stream never waits on the gate ACT->DVE chain."""
                vs = slice(c * NSUB, (c + 1) * NSUB)
                h1p = ps_main.tile([128, RC], dtf, name="ps", tag="ps")
                for k in range(KT):
                    nc.tensor.matmul(h1p[:], w1o[k][:], st[k][c][:],
                                     start=(k == 0), stop=(k == KT - 1))
                h1 = tp.tile([128, RC], dtb, name="h1", tag="h1")
                nc.vector.tensor_scalar(
                    h1[:], h1p[:], b1o[:, 0:1], 0.0, OP.add, OP.max
                )
                hhp = ps_main.tile([128, RC], dtf, name="ps", tag="ps")
                for k in range(KT):
                    nc.tensor.matmul(hhp[:], w1h[k][:], st[k][c][:],
                                     start=(k == 0), stop=(k == KT - 1))
                hh = tp.tile([128, RC], dtb, name="hh", tag="hh")
                nc.vector.tensor_scalar(
                    hh[:], hhp[:], b1h[:, 0:1], 0.0, OP.add, OP.max
                )
                # the N=1 head outputs land in spare columns of the already
                # consumed h1p/hhp psum tiles — no extra PSUM banks needed
                ovp = h1p[:, RC - NSUB:RC]
                hvp = hhp[:, RC - NSUB:RC]
                for s in range(NSUB):
                    ss = slice(s * 128, (s + 1) * 128)
                    nc.tensor.matmul(ovp[:, s:s + 1], h1[:, ss], w23[:],
                                     start=True, stop=True)
                    nc.tensor.matmul(hvp[:, s:s + 1], hh[:, ss], wh2[:],
                                     start=True, stop=True)

                # halting chain on this chunk's 4 columns (fp32)
                outv = tp.tile([128, NSUB], dtf, name="outv", tag="outv")
                nc.scalar.activation(outv[:], ovp[:], AF.Sigmoid, bias=b23[:, 0:1])
                halt = tp.tile([128, NSUB], dtf, name="halt", tag="halt")
                nc.scalar.activation(halt[:], hvp[:], AF.Sigmoid, bias=bh2[:, 0:1])
                halt_m = tp.tile([128, NSUB], dtf, name="halt_m", tag="halt_m")
                nc.vector.tensor_mul(halt_m[:], halt[:], active[:, vs])
                p_new = tp.tile([128, NSUB], dtf, name="p_new", tag="p_new")
                nc.vector.tensor_add(p_new[:], p_sum[:, vs], halt_m[:])
                fin = tp.tile([128, NSUB], dtf, name="fin", tag="fin")
                if t == MAX_ITER - 1:
                    nc.vector.memset(fin[:], 1.0)
                else:
                    nc.vector.tensor_single_scalar(fin[:], p_new[:], THR, OP.is_ge)
                adj = tp.tile([128, NSUB], dtf, name="adj", tag="adj")
                nc.vector.tensor_mul(adj[:], active[:, vs], fin[:])
                negt = tp.tile([128, NSUB], dtf, name="negt", tag="negt")
                nc.vector.scalar_tensor_tensor(
                    negt[:], p_new[:], 1.0, adj[:], OP.subtract, OP.mult
                )
                halt_adj = tp.tile([128, NSUB], dtf, name="halt_adj", tag="halt_adj")
                nc.vector.tensor_sub(halt_adj[:], halt_m[:], negt[:])
                nc.vector.tensor_sub(p_sum[:, vs], p_new[:], negt[:])
                wout = tp.tile([128, NSUB], dtf, name="wout", tag="wout")
                nc.vector.tensor_mul(wout[:], outv[:], halt_adj[:])
                nc.vector.tensor_add(acc[:, vs], acc[:, vs], wout[:])
                nc.vector.tensor_sub(active[:, vs], active[:, vs], adj[:])

            # iteration-major: keeps the distance between a chunk's state
            # write (DVE) and its next-iteration read (PE) at NCH units so
            # the in-order PE stream never stalls on it; heads trail the
            # gates by one unit for the same reason
            # emission is rotated by one nt-block: each unit's heads are
            # followed in the PE stream by the NEXT unit's first nt-block,
            # i.e. the heads' wait on the last gate block's ACT->DVE state
            # chain is covered by ~20 independent matmuls
            units = [(c, t) for t in range(T) for c in range(NCH)]
            nt_block(*units[0], 0)
            for i, (c, t) in enumerate(units):
                for n in range(1, KT):
                    nt_block(c, t, n)
                if i + 1 < len(units):
                    nt_block(*units[i + 1], 0)
                unit_heads(c, t)

            # ---- outputs ----
            # acc[p, col] is row 128*col + p; transpose 32x32 blocks so DRAM
            # write is contiguous
            accT = pp.tile([32, 128], dtf, name="accT", tag="accT")
            for b in range(4):
                nc.vector.transpose(
                    accT[0:32, b * 32:(b + 1) * 32],
                    acc[b * 32:(b + 1) * 32, 0:32],
                )
            nc.sync.dma_start(
                acc_d[:].rearrange("(a b) -> a b", a=32), accT[:]
            )
            act_red = pp.tile([128, 1], dtf, name="act_red", tag="act_red")
            nc.vector.reduce_sum(act_red[:], active[:], axis=mybir.AxisListType.X)
            nc.sync.dma_start(act_d[:], act_red[:])

    return nc


def _prep_shared(inputs):
    bf = ml_dtypes.bfloat16
    f32 = np.float32
    d = {k: np.asarray(v, dtype=f32) for k, v in inputs.items()}
    shared = {}
    for g in GATES:
        shared[f"wh_{g}"] = np.ascontiguousarray(d[f"W{g}_h"]).astype(bf)
        shared[f"wxb_{g}"] = np.ascontiguousarray(
            np.vstack([d[f"W{g}_x"], (d[f"b{g}_x"] + d[f"b{g}_h"])[None, :]])
        ).astype(bf)
    shared["w1o"] = np.ascontiguousarray(d["out_W1"]).astype(bf)
    shared["w1h"] = np.ascontiguousarray(d["halt_W1"]).astype(bf)
    shared["b1o"] = np.ascontiguousarray(d["out_b1"][:, None])
    shared["b1h"] = np.ascontiguousarray(d["halt_b1"][:, None])
    w23 = (d["out_W2"].astype(np.float64) @ d["out_W3"].astype(np.float64))
    shared["w23"] = np.ascontiguousarray(w23.astype(f32)).astype(bf)
    shared["wh2"] = np.ascontiguousarray(d["halt_W2"]).astype(bf)
    b23 = np.float32((d["out_b2"].astype(np.float64) @ d["out_W3"].astype(np.float64))[0]
                     + d["out_b3"][0])
    bh2 = np.float32(d["halt_b2"][0])
    shared["b23v"] = np.full((128, 1), b23, dtype=f32)
    shared["bh2v"] = np.full((128, 1), bh2, dtype=f32)
    x = d["x"]
    xa = np.vstack([x.T, np.ones((1, B), f32)]).astype(bf)  # [3, B]
    return shared, xa


def _run(nc, shared, xa, trace=False):
    from concourse.bass_utils import run_bass_kernel_spmd

    in_maps = []
    for i in range(NCORES):
        m = dict(shared)
        m["xa"] = np.ascontiguousarray(xa[:, i * BS:(i + 1) * BS])
        in_maps.append(m)
    return run_bass_kernel_spmd(
        nc, in_maps, core_ids=list(range(NCORES)), trace=trace
    )


def _get_nc(T):
    key = ("nc", T)
    if key not in _cache:
        _cache[key] = _build(T)
    return _cache[key]


def kernel(**inputs):
    shared, xa = _prep_shared(inputs)
    res = _run(_get_nc(3), shared, xa)
    accs = [res.results[i]["acc_out"] for i in range(NCORES)]
    n_active = sum(float(res.results[i]["act_out"].sum()) for i in range(NCORES))
    if n_active > 0.5:
        # fallback: some rows did not halt within 3 iterations — run the full
        # 32-iteration recurrence (matches the reference exactly)
        res = _run(_get_nc(MAX_ITER), shared, xa)
        accs = [res.results[i]["acc_out"] for i in range(NCORES)]
    out = np.concatenate(accs).reshape(B, 1).astype(np.float32)
    return out


# revision 31
# speedup vs baseline: 1.2696x; 1.0277x over previous
"""ACT-LSTM (adaptive computation time) forward pass on 8 TRN2 NeuronCores.

Strategy
--------
Pure data parallel: batch (32768 rows) is split into 8 shards of 4096 rows;
every core runs the full recurrence on its shard with replicated weights.

The halting dynamics of this network guarantee p_sum crosses the 1-eps
threshold for every row within 3 iterations (per-iteration halt prob is
sigmoid(..) >= ~0.47, so after 3 steps p >= ~1.4 >> 0.999).  The main kernel
therefore runs T=3 iterations and also reports the number of still-active
rows; in the (practically impossible) event rows remain active, a full
32-iteration kernel is built lazily and used instead.

On-chip layout: every [rows, H] tensor is stored transposed as a grid of
[128, 512] tiles (H on partitions, rows on free dim), which makes the whole
recurrence transpose-free: matmuls are weight-stationary
(out[n, r] = sum_k W[k, n] * state_T[k, r]), gate activations read PSUM
directly, and the elementwise cell/state updates are layout-agnostic.
Row-vector state (p_sum / active / acc) lives as [128, 32] tiles
(row = 128*col + partition), produced directly in that layout by the final
head matmuls (lhsT = hidden activations, N=1).

All matmul operands are bf16 (fp32 PSUM accumulate); the halting vector
chain is fp32.  Host-side simulation vs the fp32 reference: max elementwise
relative error ~3e-4.
"""

import numpy as np
import ml_dtypes

NCORES = 8
B = 32768
BS = B // NCORES          # rows per core
H = 512
KT = H // 128             # 4 partition tiles of the hidden dim
RC = 512                  # row-chunk (matmul moving free dim / PSUM bank)
NCH = BS // RC            # 8 row chunks
NSUB = RC // 128          # 4 sub-chunks of 128 rows per chunk
NCOL = NCH * NSUB         # 32 columns of the [128, 32] row-vector tiles
MAX_ITER = 32
THR = float(np.float32(1.0) - np.float32(1e-3))
GATES = ("i", "f", "c", "o")

_cache = {}


def _make_tc_class():
    import concourse.mybir as mybir
    import concourse.tile as tile
    from concourse.vector_clock import ScopedClock

    class _TC(tile.TileContext):
        """TileContext adjusted for this toolchain's walrus, which encodes at
        most one sync wait and one sem update per instruction (and none on
        Drain).  Extra syncs are spread over adjacent no-ops on the same
        engine (safe: engine streams issue in order), and the exit barrier
        (whose eq-waits are unencodable) is replaced by explicit per-sem
        wait_ge instructions + plain drains.  Semaphores start zeroed at NEFF
        load and we load freshly per run, so no exit sem-clear is needed."""

        def _drain_and_barrier(self, tick_clock, wait_clock):
            nc = self.nc
            probe = mybir.InstNoOp(name="tile_exit_wait_probe", ins=[], outs=[])
            probe.engine = mybir.EngineType.SP
            wait_clock.add_sem_waits(
                probe, ScopedClock({None: tick_clock.global_clock})
            )
            handles = {h.name: h for h in wait_clock.sems.allocated().values()}
            si = probe.sync_info
            if si is not None:
                # only DMA-queue sems need the explicit exit wait (their
                # engine-side drain doesn't cover in-flight completions of
                # queues other engines triggered); compute-engine ticks are
                # implied by each engine running its stream to the end
                for w in si.on_wait:
                    if "DMA" in w.ant_name:
                        nc.sync.wait_ge(handles[w.ant_name], w.wait_value)
            for _, eng in nc.engines.items():
                eng.drain()
            popped = nc._tile_sem_poison_stack.pop()
            assert popped is self._sem_poison

        def _lower_ordered_insts(self, ordered):
            nc = self.nc

            def mknop(engine, wait=None, update=None):
                n = mybir.InstNoOp(
                    name=nc.get_next_instruction_name(), ins=[], outs=[]
                )
                n.engine = engine
                n.bass_nofuse = True
                n.sync_info = mybir.SyncInfo(
                    on_wait=[wait] if wait is not None else [],
                    on_update=[update] if update is not None else [],
                )
                return n

            for bb, insts in ordered.items():
                out = []
                for inst in insts:
                    si = inst.sync_info
                    if si is None:
                        out.append(inst)
                        continue
                    waits = list(si.on_wait)
                    ups = list(si.on_update)
                    for w in waits:
                        assert w.wait_mode == "sem-ge-imm", w
                    if isinstance(inst, mybir.InstDrain):
                        pre, keepw = waits, []
                        keepu, post = [], ups
                    else:
                        pre, keepw = waits[:-1], waits[-1:]
                        keepu, post = ups[:1], ups[1:]
                    if pre or post:
                        for w in pre:
                            out.append(mknop(inst.engine, wait=w))
                        inst.sync_info = mybir.SyncInfo(
                            on_wait=keepw, on_update=keepu
                        )
                        out.append(inst)
                        for u in post:
                            out.append(mknop(inst.engine, update=u))
                    else:
                        out.append(inst)
                ordered[bb] = out
            super()._lower_ordered_insts(ordered)

    return _TC


def _build(T):
    """Build the Bass graph for T recurrence iterations (is_last at t==31)."""
    import concourse.bass as bass
    import concourse.mybir as mybir

    dtf = mybir.dt.float32
    dtb = mybir.dt.bfloat16
    AF = mybir.ActivationFunctionType
    OP = mybir.AluOpType
    TC = _make_tc_class()

    nc = bass.Bass()

    xa_d = nc.declare_dram_parameter("xa", [3, BS], dtb, isOutput=False)
    wh_d = {g: nc.declare_dram_parameter(f"wh_{g}", [H, H], dtb, isOutput=False)
            for g in GATES}
    wxb_d = {g: nc.declare_dram_parameter(f"wxb_{g}", [3, H], dtb, isOutput=False)
             for g in GATES}
    w1o_d = nc.declare_dram_parameter("w1o", [H, 128], dtb, isOutput=False)
    w1h_d = nc.declare_dram_parameter("w1h", [H, 128], dtb, isOutput=False)
    b1o_d = nc.declare_dram_parameter("b1o", [128, 1], dtf, isOutput=False)
    b1h_d = nc.declare_dram_parameter("b1h", [128, 1], dtf, isOutput=False)
    w23_d = nc.declare_dram_parameter("w23", [128, 1], dtb, isOutput=False)
    wh2_d = nc.declare_dram_parameter("wh2", [128, 1], dtb, isOutput=False)
    b23_d = nc.declare_dram_parameter("b23v", [128, 1], dtf, isOutput=False)
    bh2_d = nc.declare_dram_parameter("bh2v", [128, 1], dtf, isOutput=False)
    acc_d = nc.declare_dram_parameter("acc_out", [BS], dtf, isOutput=True)
    act_d = nc.declare_dram_parameter("act_out", [128, 1], dtf, isOutput=True)

    with TC(nc) as tc:
        with (
            tc.tile_pool(name="persist", bufs=1) as pp,
            tc.tile_pool(name="trans", bufs=2) as tp,
            tc.tile_pool(name="ps_main", bufs=6, space="PSUM") as ps_main,
            tc.tile_pool(name="ps_vec", bufs=1, space="PSUM") as ps_vec,
        ):
            # ---- load weights / inputs ----
            # x-projection operands first: t0 only needs these (the 2.1MB of
            # hidden weights are loaded last, overlapped with t0 compute).
            # They are replicated at partition offsets 0/32/64/96 so the four
            # gates' K=3 matmuls run concurrently in distinct PE row groups
            # (tile_position row tiling).
            xa_rep = pp.tile([128, BS], dtb, name="xa_rep", tag="xa_rep")
            wxb_rep = pp.tile([128, H], dtb, name="wxb_rep", tag="wxb_rep")
            for gi, g in enumerate(GATES):
                nc.sync.dma_start(xa_rep[32 * gi:32 * gi + 3, :], xa_d[:])
                nc.sync.dma_start(wxb_rep[32 * gi:32 * gi + 3, :], wxb_d[g][:])
            w1o, w1h = [], []
            for k in range(KT):
                a = pp.tile([128, 128], dtb, name=f"w1o{k}", tag=f"w1o{k}")
                nc.sync.dma_start(a[:], w1o_d[k * 128:(k + 1) * 128, :])
                w1o.append(a)
                b = pp.tile([128, 128], dtb, name=f"w1h{k}", tag=f"w1h{k}")
                nc.sync.dma_start(b[:], w1h_d[k * 128:(k + 1) * 128, :])
                w1h.append(b)
            b1o = pp.tile([128, 1], dtf, name="b1o", tag="b1o")
            nc.sync.dma_start(b1o[:], b1o_d[:])
            b1h = pp.tile([128, 1], dtf, name="b1h", tag="b1h")
            nc.sync.dma_start(b1h[:], b1h_d[:])
            w23 = pp.tile([128, 1], dtb, name="w23", tag="w23")
            nc.sync.dma_start(w23[:], w23_d[:])
            wh2 = pp.tile([128, 1], dtb, name="wh2", tag="wh2")
            nc.sync.dma_start(wh2[:], wh2_d[:])
            b23 = pp.tile([128, 1], dtf, name="b23", tag="b23")
            nc.sync.dma_start(b23[:], b23_d[:])
            bh2 = pp.tile([128, 1], dtf, name="bh2", tag="bh2")
            nc.sync.dma_start(bh2[:], bh2_d[:])
            wh = {}
            for g in GATES:
                for k in range(KT):
                    t = pp.tile([128, H], dtb, name=f"wh_{g}{k}", tag=f"wh_{g}{k}")
                    nc.sync.dma_start(t[:], wh_d[g][k * 128:(k + 1) * 128, :])
                    wh[(g, k)] = t

            # ---- persistent recurrent state ----
            st = [[pp.tile([128, RC], dtb, name=f"st{k}_{c}", tag=f"st{k}_{c}") for c in range(NCH)]
                  for k in range(KT)]
            cl = [[pp.tile([128, RC], dtb, name=f"cl{k}_{c}", tag=f"cl{k}_{c}") for c in range(NCH)]
                  for k in range(KT)]
            p_sum = pp.tile([128, NCOL], dtf, name="p_sum", tag="p_sum")
            active = pp.tile([128, NCOL], dtf, name="active", tag="active")
            acc = pp.tile([128, NCOL], dtf, name="acc", tag="acc")
            nc.vector.memset(p_sum[:], 0.0)
            nc.vector.memset(active[:], 1.0)
            nc.vector.memset(acc[:], 0.0)

            AFG = {"i": AF.Sigmoid, "f": AF.Sigmoid, "c": AF.Tanh, "o": AF.Sigmoid}

            def nt_block(c, t, n):
                """Gate matmuls + activations + cell/state update for one
                (row-chunk, iteration, H-slice) block."""
                cs = slice(c * RC, (c + 1) * RC)
                gates_t = GATES if t > 0 else ("i", "c", "o")
                ns = slice(n * 128, (n + 1) * 128)
                psg = {}
                for g in gates_t:
                    gi = GATES.index(g)
                    rs = slice(32 * gi, 32 * gi + 3)
                    ps = ps_main.tile([128, RC], dtf, name="ps", tag="ps")
                    nc.tensor.matmul(
                        ps[:], wxb_rep[rs, ns], xa_rep[rs, cs],
                        start=True, stop=(t == 0),
                        tile_position=(32 * gi, 0),
                    )
                    psg[g] = ps
                gsb = {}
                for g in gates_t:
                    ps = psg[g]
                    if t > 0:
                        for k in range(KT):
                            nc.tensor.matmul(
                                ps[:], wh[(g, k)][:, ns], st[k][c][:],
                                start=False, stop=(k == KT - 1),
                            )
                    gt = tp.tile([128, RC], dtb, name=f"g_{g}", tag=f"g_{g}")
                    nc.scalar.activation(gt[:], ps[:], AFG[g])
                    gsb[g] = gt
                if t == 0:
                    nc.vector.tensor_mul(cl[n][c][:], gsb["i"][:], gsb["c"][:])
                else:
                    t1 = tp.tile([128, RC], dtb, name="t1", tag="t1")
                    nc.vector.tensor_mul(t1[:], gsb["f"][:], cl[n][c][:])
                    t2 = tp.tile([128, RC], dtb, name="t2", tag="t2")
                    nc.vector.tensor_mul(t2[:], gsb["i"][:], gsb["c"][:])
                    nc.vector.tensor_add(cl[n][c][:], t1[:], t2[:])
                tnc = tp.tile([128, RC], dtb, name="tnc", tag="tnc")
                nc.scalar.activation(tnc[:], cl[n][c][:], AF.Tanh)
                nc.vector.tensor_mul(st[n][c][:], gsb["o"][:], tnc[:])

            def unit_heads(c, t):
                """Output/halt heads + this chunk's slice of the halting
                chain.  Emitted one unit after unit_gates(c, t) so the PE
                stream never waits on the gate ACT->DVE chain."""
                vs = slice(c * NSUB, (c + 1) * NSUB)
                h1p = ps_main.tile([128, RC], dtf, name="ps", tag="ps")
                for k in range(KT):
                    nc.tensor.matmul(h1p[:], w1o[k][:], st[k][c][:],
                                     start=(k == 0), stop=(k == KT - 1))
                h1 = tp.tile([128, RC], dtb, name="h1", tag="h1")
                nc.vector.tensor_scalar(
                    h1[:], h1p[:], b1o[:, 0:1], 0.0, OP.add, OP.max
                )
                hhp = ps_main.tile([128, RC], dtf, name="ps", tag="ps")
                for k in range(KT):
                    nc.tensor.matmul(hhp[:], w1h[k][:], st[k][c][:],
                                     start=(k == 0), stop=(k == KT - 1))
                hh = tp.tile([128, RC], dtb, name="hh", tag="hh")
                nc.vector.tensor_scalar(
                    hh[:], hhp[:], b1h[:, 0:1], 0.0, OP.add, OP.max
                )
                ovp = ps_main.tile([128, NSUB], dtf, name="ovp", tag="ov",
                                   bufs=1)
                hvp = ps_main.tile([128, NSUB], dtf, name="hvp", tag="hv",
                                   bufs=1)
                for s in range(NSUB):
                    ss = slice(s * 128, (s + 1) * 128)
                    nc.tensor.matmul(ovp[:, s:s + 1], h1[:, ss], w23[:],
                                     start=True, stop=True)
                    nc.tensor.matmul(hvp[:, s:s + 1], hh[:, ss], wh2[:],
                                     start=True, stop=True)

                # halting chain on this chunk's 4 columns (fp32)
                outv = tp.tile([128, NSUB], dtf, name="outv", tag="outv")
                nc.scalar.activation(outv[:], ovp[:], AF.Sigmoid, bias=b23[:, 0:1])
                halt = tp.tile([128, NSUB], dtf, name="halt", tag="halt")
                nc.scalar.activation(halt[:], hvp[:], AF.Sigmoid, bias=bh2[:, 0:1])
                halt_m = tp.tile([128, NSUB], dtf, name="halt_m", tag="halt_m")
                nc.vector.tensor_mul(halt_m[:], halt[:], active[:, vs])
                p_new = tp.tile([128, NSUB], dtf, name="p_new", tag="p_new")
                nc.vector.tensor_add(p_new[:], p_sum[:, vs], halt_m[:])
                fin = tp.tile([128, NSUB], dtf, name="fin", tag="fin")
                if t == MAX_ITER - 1:
                    nc.vector.memset(fin[:], 1.0)
                else:
                    nc.vector.tensor_single_scalar(fin[:], p_new[:], THR, OP.is_ge)
                adj = tp.tile([128, NSUB], dtf, name="adj", tag="adj")
                nc.vector.tensor_mul(adj[:], active[:, vs], fin[:])
                negt = tp.tile([128, NSUB], dtf, name="negt", tag="negt")
                nc.vector.scalar_tensor_tensor(
                    negt[:], p_new[:], 1.0, adj[:], OP.subtract, OP.mult
                )
                halt_adj = tp.tile([128, NSUB], dtf, name="halt_adj", tag="halt_adj")
                nc.vector.tensor_sub(halt_adj[:], halt_m[:], negt[:])
                nc.vector.tensor_sub(p_sum[:, vs], p_new[:], negt[:])
                wout = tp.tile([128, NSUB], dtf, name="wout", tag="wout")
                nc.vector.tensor_mul(wout[:], outv[:], halt_adj[:])
                nc.vector.tensor_add(acc[:, vs], acc[:, vs], wout[:])
                nc.vector.tensor_sub(active[:, vs], active[:, vs], adj[:])

            # iteration-major: keeps the distance between a chunk's state
            # write (DVE) and its next-iteration read (PE) at NCH units so
            # the in-order PE stream never stalls on it; heads trail the
            # gates by one unit for the same reason
            # emission is rotated by one nt-block: each unit's heads are
            # followed in the PE stream by the NEXT unit's first nt-block,
            # i.e. the heads' wait on the last gate block's ACT->DVE state
            # chain is covered by ~20 independent matmuls
            units = [(c, t) for t in range(T) for c in range(NCH)]
            nt_block(*units[0], 0)
            for i, (c, t) in enumerate(units):
                for n in range(1, KT):
                    nt_block(c, t, n)
                if i + 1 < len(units):
                    nt_block(*units[i + 1], 0)
                unit_heads(c, t)

            # ---- outputs ----
            # acc[p, col] is row 128*col + p; transpose 32x32 blocks so DRAM
            # write is contiguous
            accT = pp.tile([32, 128], dtf, name="accT", tag="accT")
            for b in range(4):
                nc.vector.transpose(
                    accT[0:32, b * 32:(b + 1) * 32],
                    acc[b * 32:(b + 1) * 32, 0:32],
                )
            nc.sync.dma_start(
                acc_d[:].rearrange("(a b) -> a b", a=32), accT[:]
            )
            act_red = pp.tile([128, 1], dtf, name="act_red", tag="act_red")
            nc.vector.reduce_sum(act_red[:], active[:], axis=mybir.AxisListType.X)
            nc.sync.dma_start(act_d[:], act_red[:])

    return nc


def _prep_shared(inputs):
    bf = ml_dtypes.bfloat16
    f32 = np.float32
    d = {k: np.asarray(v, dtype=f32) for k, v in inputs.items()}
    shared = {}
    for g in GATES:
        shared[f"wh_{g}"] = np.ascontiguousarray(d[f"W{g}_h"]).astype(bf)
        shared[f"wxb_{g}"] = np.ascontiguousarray(
            np.vstack([d[f"W{g}_x"], (d[f"b{g}_x"] + d[f"b{g}_h"])[None, :]])
        ).astype(bf)
    shared["w1o"] = np.ascontiguousarray(d["out_W1"]).astype(bf)
    shared["w1h"] = np.ascontiguousarray(d["halt_W1"]).astype(bf)
    shared["b1o"] = np.ascontiguousarray(d["out_b1"][:, None])
    shared["b1h"] = np.ascontiguousarray(d["halt_b1"][:, None])
    w23 = (d["out_W2"].astype(np.float64) @ d["out_W3"].astype(np.float64))
    shared["w23"] = np.ascontiguousarray(w23.astype(f32)).astype(bf)
    shared["wh2"] = np.ascontiguousarray(d["halt_W2"]).astype(bf)
    b23 = np.float32((d["out_b2"].astype(np.float64) @ d["out_W3"].astype(np.float64))[0]
                     + d["out_b3"][0])
    bh2 = np.float32(d["halt_b2"][0])
    shared["b23v"] = np.full((128, 1), b23, dtype=f32)
    shared["bh2v"] = np.full((128, 1), bh2, dtype=f32)
    x = d["x"]
    xa = np.vstack([x.T, np.ones((1, B), f32)]).astype(bf)  # [3, B]
    return shared, xa


def _run(nc, shared, xa, trace=False):
    from concourse.bass_utils import run_bass_kernel_spmd

    in_maps = []
    for i in range(NCORES):
        m = dict(shared)
        m["xa"] = np.ascontiguousarray(xa[:, i * BS:(i + 1) * BS])
        in_maps.append(m)
    return run_bass_kernel_spmd(
        nc, in_maps, core_ids=list(range(NCORES)), trace=trace
    )


def _get_nc(T):
    key = ("nc", T)
    if key not in _cache:
        _cache[key] = _build(T)
    return _cache[key]


def kernel(**inputs):
    shared, xa = _prep_shared(inputs)
    res = _run(_get_nc(3), shared, xa)
    accs = [res.results[i]["acc_out"] for i in range(NCORES)]
    n_active = sum(float(res.results[i]["act_out"].sum()) for i in range(NCORES))
    if n_active > 0.5:
        # fallback: some rows did not halt within 3 iterations — run the full
        # 32-iteration recurrence (matches the reference exactly)
        res = _run(_get_nc(MAX_ITER), shared, xa)
        accs = [res.results[i]["acc_out"] for i in range(NCORES)]
    out = np.concatenate(accs).reshape(B, 1).astype(np.float32)
    return out
